# revision 2
# baseline (speedup 1.0000x reference)
"""Trainium2 Bass kernel for Qwen2-style causal self-attention (GQA + RoPE), v5.

Geometry: B=4 seqs x S=2048 tokens, 14 Q heads / 2 KV heads, D=64, HID=896.
Sharding: 8 cores = 4 sequences x 2 head-groups (7 Q heads + 1 KV head each).
Each core computes its sequence's QKV projections (its head shard), RoPE,
causal attention, and a partial o_proj (448 input dims); the host sums the
two partials per sequence.

Pipeline design (cost-model-driven):
  - ScalarE is the bottleneck engine: it does exp and nothing else
    (140 calls x [128,1024] ~ 145us).  Everything else is organized to
    keep its input (score PSUM tiles) always available.
  - all matmul operands are bf16 (host-converted, DMA'd straight into
    matmuls); full hidden state is resident in SBUF (28KB/partition).
  - PV runs fp8e4 DoubleRow (two 128-token key blocks per instruction at
    0.5 cycles/row); probs are quantized to fp8 by the exp activation
    itself.  Chunk 0 queries average few keys, so its PV runs in bf16
    (fp8 there dominated the output error).
  - causal masking on GPSIMD affine_select, restricted to the affected
    column span of each diagonal block.
  - softmax normalization per head stays on-chip: reciprocal of the
    [V|1] ones-column sums at partition 64, broadcast down with a K=1
    matmul into the spare rows of the same PSUM tile, then one DVE mul.
  - heads are re-paired for o_proj (even head -> partitions 0:64, odd ->
    64:128) with identity matmuls into the same PSUM tile, then one DVE
    copy into the otl pair tile; o_proj contracts K=128 per pair.
  - proj(c+1) and oproj(c-1) are sliced into attention(c)'s head loop so
    every engine FIFO (esp. DVE) alternates between pipelines instead of
    head-of-line blocking on a whole phase.
  - PSUM: psS (scores) 2x2 banks, pspv 2x1, psA (proj) 1, po (o_proj) 1;
    proj(0) and the final o_proj borrow the idle psS buffers instead.
"""

import numpy as np
from contextlib import ExitStack

import concourse.bacc as bacc
import concourse.bass as bass
import concourse.mybir as mybir
import concourse.tile as tile
from concourse.bass_utils import run_bass_kernel_spmd

import ml_dtypes

_BF16 = ml_dtypes.bfloat16

B, S = 4, 2048
H, KV, D = 14, 2, 64
HID = H * D  # 896
THETA = 1000000.0
G = 2  # tensor-parallel head groups
HG = H // G  # 7 q heads per group
NQ = HG * D  # 448
NQK = NQ + D  # 512 = q dims + k dims per group
KBLK = HID // 128  # 7 hid blocks
NSLAB = NQK // 128  # 4 slabs of the roped qk output
NTOK = S // 128  # 16 token blocks
NCHUNK = S // 512  # 4 token chunks
N_CORES = 8

F32 = mybir.dt.float32
BF16 = mybir.dt.bfloat16
FP8 = mybir.dt.float8e4
AF = mybir.ActivationFunctionType
ALU = mybir.AluOpType

PV_MODE = "fp8dr"  # "fp8dr" | "bf16"

_CACHE = {}


def _build():
    nc = bacc.Bacc("TRN2", target_bir_lowering=False, debug=False)

    hT = nc.dram_tensor("hT", [KBLK, 128, S], FP8, kind="ExternalInput")
    wqk = nc.dram_tensor("wqk", [KBLK, 128, NQK], FP8, kind="ExternalInput")
    wv = nc.dram_tensor("wv", [KBLK, 128, D], FP8, kind="ExternalInput")
    hb0 = nc.dram_tensor("hb0", [KBLK, 128, 512], BF16, kind="ExternalInput")
    wqkb = nc.dram_tensor("wqkb", [KBLK, 128, NQK], BF16, kind="ExternalInput")
    wvb = nc.dram_tensor("wvb", [KBLK, 128, D], BF16, kind="ExternalInput")
    bqk = nc.dram_tensor("bqk", [128, NSLAB], F32, kind="ExternalInput")
    vb = nc.dram_tensor("vb", [1, D + 2], BF16, kind="ExternalInput")
    ow = nc.dram_tensor("ow", [4, 128, HID], FP8, kind="ExternalInput")
    owb = nc.dram_tensor("owb", [4, 128, HID], BF16, kind="ExternalInput")
    cosf = nc.dram_tensor("cosf", [128, S], BF16, kind="ExternalInput")
    sinpat = nc.dram_tensor("sinpat", [128, S], BF16, kind="ExternalInput")
    perm = nc.dram_tensor("perm", [128, 320], BF16, kind="ExternalInput")
    out = nc.dram_tensor("out", [S, HID], BF16, kind="ExternalOutput")

    vdt = FP8 if PV_MODE == "fp8dr" else BF16

    with tile.TileContext(nc) as tc, ExitStack() as ctx:
        P = ctx.enter_context(tc.tile_pool(name="persist", bufs=1))
        RT = ctx.enter_context(tc.tile_pool(name="rt", bufs=2))
        PT = ctx.enter_context(tc.tile_pool(name="pt", bufs=3))
        RZ = ctx.enter_context(tc.tile_pool(name="rz", bufs=2))
        ON = ctx.enter_context(tc.tile_pool(name="on", bufs=3))
        OL = ctx.enter_context(tc.tile_pool(name="ol", bufs=2))
        OB = ctx.enter_context(tc.tile_pool(name="ob", bufs=2))
        PS1 = ctx.enter_context(tc.tile_pool(name="ps1", bufs=2, space="PSUM"))
        PS2 = ctx.enter_context(tc.tile_pool(name="ps2", bufs=2, space="PSUM"))
        PS3 = ctx.enter_context(tc.tile_pool(name="ps3", bufs=2, space="PSUM"))

        # ---- persistent tiles ----
        qk_sb = [P.tile([128, S], BF16, tag=f"qk{s}", name=f"qk{s}") for s in range(NSLAB)]
        kTd = P.tile([128, S], BF16, tag="kTd")
        cos_sb = P.tile([128, S], BF16, tag="cos")
        sin_sb = P.tile([128, S], BF16, tag="sin")
        bqk_sb = P.tile([128, NSLAB], F32, tag="bqk")
        wqk_sb = P.tile([128, KBLK, NQK], FP8, tag="wqk_sb")
        wv_sb = P.tile([128, KBLK, D], FP8, tag="wv_sb")
        ow_sb = P.tile([128, 4, HID], FP8, tag="ow_sb")
        owb_sb = P.tile([128, 4, HID], BF16, tag="owb_sb")
        vb_sb = P.tile([1, D + 2], BF16, tag="vb_sb")
        ones1 = P.tile([1, 128], BF16, tag="ones1")
        perm_sb = P.tile([128, 320], BF16, tag="perm_sb")
        v8 = P.tile([128, NTOK, 80], vdt, tag="v8")
        vbf = P.tile([128, 4, 80], BF16, tag="vbf")  # chunk-0 V in bf16
        h_sb = P.tile([128, KBLK, S], FP8, tag="h_sb")  # full hidden, resident
        hb0_sb = P.tile([128, KBLK, 512], BF16, tag="hb0_sb")  # chunk-0 hidden
        wqkb_sb = P.tile([128, KBLK, NQK], BF16, tag="wqkb_sb")
        wvb_sb = P.tile([128, KBLK, D], BF16, tag="wvb_sb")
        ones64 = P.tile([65, D], BF16, tag="ones64")
        ones512 = P.tile([1, 512], BF16, tag="ones512")

        # warm the PE p-state while the startup DMAs stream: the cost model
        # runs matmuls at half rate until ~3us of continuous PE activity
        nc.vector.memset(ones1, 1.0)
        nc.vector.memset(ones512, 1.0)
        for _w in range(8):
            wps = PS3.tile([128, 512], F32, tag="psA", name="warm")
            nc.tensor.matmul(wps, ones1, ones512, start=True, stop=True)

        # startup loads, most critical first: proj(0) slab 3 (K) needs the
        # slab-3 weight columns + chunk-0 hidden + bias + perm + cos/sin
        nc.sync.dma_start(
            out=wqkb_sb[:, :, 384:512],
            in_=wqkb[:, :, 384:512].rearrange("k p m -> p k m"),
        )
        nc.sync.dma_start(
            out=wqkb_sb[:, :, 0:128],
            in_=wqkb[:, :, 0:128].rearrange("k p m -> p k m"),
        )
        nc.sync.dma_start(
            out=hb0_sb[:, 0:4, :], in_=hb0[0:4, :, :].rearrange("k p t -> p k t")
        )
        nc.sync.dma_start(
            out=hb0_sb[:, 4:KBLK, :],
            in_=hb0[4:KBLK, :, :].rearrange("k p t -> p k t"),
        )
        nc.sync.dma_start(out=bqk_sb, in_=bqk[:, :])
        nc.sync.dma_start(out=perm_sb, in_=perm[:, :])
        nc.sync.dma_start(out=cos_sb[:, 0:512], in_=cosf[:, 0:512])
        nc.sync.dma_start(out=sin_sb[:, 0:512], in_=sinpat[:, 0:512])
        nc.sync.dma_start(out=cos_sb[:, 512:S], in_=cosf[:, 512:S])
        nc.sync.dma_start(out=sin_sb[:, 512:S], in_=sinpat[:, 512:S])
        nc.sync.dma_start(
            out=wqkb_sb[:, :, 128:384],
            in_=wqkb[:, :, 128:384].rearrange("k p m -> p k m"),
        )
        nc.sync.dma_start(out=wvb_sb, in_=wvb[:, :, :].rearrange("k p m -> p k m"))
        nc.sync.dma_start(out=wqk_sb, in_=wqk[:, :, :].rearrange("k p m -> p k m"))
        nc.sync.dma_start(out=wv_sb, in_=wv[:, :, :].rearrange("k p m -> p k m"))
        nc.sync.dma_start(out=vb_sb, in_=vb[:, :])
        for c in range(1, NCHUNK):
            nc.sync.dma_start(
                out=h_sb[:, :, 512 * c : 512 * c + 512],
                in_=hT[:, :, 512 * c : 512 * c + 512].rearrange("k p t -> p k t"),
            )
        nc.sync.dma_start(out=ow_sb, in_=ow[:, :, :].rearrange("b p m -> p b m"))
        nc.sync.dma_start(out=owb_sb, in_=owb[:, :, :].rearrange("b p m -> p b m"))
        nc.vector.memset(ones64, 1.0)

        def psa(use_ps1):
            if use_ps1:
                return PS1.tile([128, 1024], F32, tag="psS", name="psA")[:, 0:512]
            return PS3.tile([128, 512], F32, tag="psA", name="psA")

        def proj_slab(c, s, use_ps1=False):
            t0 = 512 * c
            ps = psa(use_ps1)
            if c == 0:
                # chunk-0 keys/values feed every later chunk and its outputs
                # are single-value-dominated: full bf16 projection
                for k in range(KBLK):
                    nc.tensor.matmul(
                        ps,
                        wqkb_sb[:, k, 128 * s : 128 * s + 128],
                        hb0_sb[:, k, :],
                        start=(k == 0),
                        stop=(k == KBLK - 1),
                    )
            else:
                h_t = h_sb[:, :, t0 : t0 + 512]
                for p in range(3):  # contraction pairs of hid blocks (fp8 DR)
                    nc.tensor.matmul(
                        ps,
                        wqk_sb[:, 2 * p : 2 * p + 2, 128 * s : 128 * s + 128],
                        h_t[:, 2 * p : 2 * p + 2, :],
                        start=(p == 0),
                        stop=False,
                        perf_mode=mybir.MatmulPerfMode.DoubleRow,
                        skip_group_check=True,
                    )
                nc.tensor.matmul(
                    ps,
                    wqk_sb[:, 6, 128 * s : 128 * s + 128],
                    h_t[:, 6, :],
                    start=False,
                    stop=True,
                    skip_group_check=True,
                )
            q = qk_sb[s][:, t0 : t0 + 512]
            nc.vector.tensor_scalar_add(q, ps, bqk_sb[:, s : s + 1])
            # rotate_half via sign-folded permutation matmul (cross-partition);
            # the slab's PSUM tile is reused for the rotate and dup outputs
            nc.tensor.matmul(ps, perm_sb[:, 0:128], q, start=True, stop=True)
            r = RT.tile([128, 512], BF16, tag="qkr", name="qkr")
            nc.vector.tensor_mul(r, ps, sin_sb[:, t0 : t0 + 512])
            nc.vector.tensor_mul(q, q, cos_sb[:, t0 : t0 + 512])
            nc.vector.tensor_add(q, q, r)
            if s == NSLAB - 1:
                # duplicated roped K^T (both partition halves)
                nc.tensor.matmul(ps, perm_sb[:, 128:256], q, start=True, stop=True)
                nc.vector.tensor_copy(out=kTd[:, t0 : t0 + 512], in_=ps)

        def proj_v(c, tb, use_ps1=False):
            t0 = 512 * c
            h_t = h_sb[:, :, t0 : t0 + 512]
            t = 4 * c + tb
            psv = psa(use_ps1)[:, 0 : D + 2]
            nc.tensor.matmul(
                psv, ones1, vb_sb, start=True, stop=False, skip_group_check=True
            )
            if c == 0:
                for k in range(KBLK):
                    nc.tensor.matmul(
                        psv[:, 0:D],
                        hb0_sb[:, k, 128 * tb : 128 * tb + 128],
                        wvb_sb[:, k, :],
                        start=False,
                        stop=(k == KBLK - 1),
                        skip_group_check=True,
                    )
            else:
                for p in range(3):
                    nc.tensor.matmul(
                        psv[:, 0:D],
                        h_t[:, 2 * p : 2 * p + 2, 128 * tb : 128 * tb + 128],
                        wv_sb[:, 2 * p : 2 * p + 2, :],
                        start=False,
                        stop=False,
                        perf_mode=mybir.MatmulPerfMode.DoubleRow,
                        skip_group_check=True,
                    )
                nc.tensor.matmul(
                    psv[:, 0:D],
                    h_t[:, 6, 128 * tb : 128 * tb + 128],
                    wv_sb[:, 6, :],
                    start=False,
                    stop=True,
                    skip_group_check=True,
                )
            nc.vector.tensor_copy(out=v8[:, t, 0 : D + 2], in_=psv)
            if c == 0:
                nc.vector.tensor_copy(out=vbf[:, t, 0 : D + 2], in_=psv)

        def proj_steps(c, use_ps1=False):
            steps = [lambda s=s: proj_slab(c, s, use_ps1) for s in (3, 0, 1, 2)]
            steps += [lambda tb=tb: proj_v(c, tb, use_ps1) for tb in range(4)]
            return steps

        def oproj_tb(c, otl, tb, use_ps1=False):
            t = 4 * c + tb
            ob = OB.tile([128, HID], BF16, tag="ob", name="ob")
            for n0, n1 in ((0, 512), (512, HID)):
                if use_ps1:
                    po = PS1.tile([128, 1024], F32, tag="psS", name="po")[:, 0:512]
                else:
                    po = PS3.tile([128, 512], F32, tag="psA", name="po")
                if c > 0:
                    # fp8 DR: contract head-pair blocks two at a time (K=256)
                    for i in range(2):
                        nc.tensor.matmul(
                            po[:, 0 : n1 - n0],
                            otl[:, 2 * i : 2 * i + 2, 128 * tb : 128 * tb + 128],
                            ow_sb[:, 2 * i : 2 * i + 2, n0:n1],
                            start=(i == 0),
                            stop=(i == 1),
                            perf_mode=mybir.MatmulPerfMode.DoubleRow,
                            skip_group_check=True,
                        )
                else:
                    for pb in range(4):
                        p_n = 128 if pb < 3 else 64
                        nc.tensor.matmul(
                            po[:, 0 : n1 - n0],
                            otl[0:p_n, pb, 128 * tb : 128 * tb + 128],
                            owb_sb[0:p_n, pb, n0:n1],
                            start=(pb == 0),
                            stop=(pb == 3),
                        )
                if use_ps1 and n0 == 0:
                    nc.scalar.copy(out=ob[:, n0:n1], in_=po[:, 0 : n1 - n0])
                else:
                    nc.vector.tensor_copy(
                        out=ob[:, n0:n1], in_=po[:, 0 : n1 - n0]
                    )
            nc.sync.dma_start(out=out[128 * t : 128 * t + 128, :], in_=ob)

        def att_head(c, h, otl, state):
            t0 = 512 * c
            npair = 2 * c + 2
            use_dr = PV_MODE == "fp8dr" and c > 0
            ptdt = vdt if use_dr else BF16
            # diagonal pairs first so gpsimd masking overlaps later blocks
            jp_order = [2 * c, 2 * c + 1] + list(range(0, 2 * c))
            slab = h // 2
            row = 64 * (h % 2)
            q_ap = qk_sb[slab][row : row + 64, t0 : t0 + 512]
            pspv = PS2.tile([128, 512], F32, tag="pspv", name="pspv")
            n_pv = 0
            for jp in jp_order:
                pss = PS1.tile([128, 1024], F32, tag="psS", name="psS")
                diag_b = jp == 2 * c + 1  # blocks 4c+2/4c+3: half the columns
                # of this pair are fully above the diagonal; don't exp them
                for u in range(2):
                    j = 2 * jp + u
                    nc.tensor.matmul(
                        pss[:, 512 * u : 512 * u + 512],
                        kTd[row : row + 64, 128 * j : 128 * j + 128],
                        q_ap,
                        start=True,
                        stop=True,
                    )
                pt = PT.tile(
                    [128, 2, 512], ptdt,
                    tag="pt" if use_dr else "ptb",
                    name="pt",
                )
                if diag_b:
                    nc.gpsimd.memset(pt[:, 0, 0:256], 0.0)
                    nc.gpsimd.memset(pt[:, 1, 0:384], 0.0)
                    nc.scalar.activation(
                        out=pt[:, 0, 256:512], in_=pss[:, 256:512],
                        func=AF.Exp, scale=0.125,
                    )
                    nc.scalar.activation(
                        out=pt[:, 1, 384:512], in_=pss[:, 896:1024],
                        func=AF.Exp, scale=0.125,
                    )
                    for u in range(2):
                        off = 256 + 128 * u
                        nc.gpsimd.affine_select(
                            out=pt[:, u, off : off + 128],
                            in_=pt[:, u, off : off + 128],
                            compare_op=ALU.is_ge,
                            fill=0.0,
                            base=0,
                            channel_multiplier=-1,
                            pattern=[[1, 128]],
                        )
                else:
                    nc.scalar.activation(
                        out=pt.rearrange("p a b -> p (a b)"), in_=pss,
                        func=AF.Exp, scale=0.125,
                    )
                    for u in range(2):
                        j = 2 * jp + u
                        if j >= 4 * c:  # diagonal block: zero out k > q
                            span = 128 * (j - 4 * c) + 128
                            nc.gpsimd.affine_select(
                                out=pt[:, u, 0:span],
                                in_=pt[:, u, 0:span],
                                compare_op=ALU.is_ge,
                                fill=0.0,
                                base=t0 - 128 * j,
                                channel_multiplier=-1,
                                pattern=[[1, span]],
                            )
                n_pv += 1
                if use_dr:
                    nc.tensor.matmul(
                        pspv[0 : D + 2, :],
                        v8[:, 2 * jp : 2 * jp + 2, 0 : D + 2],
                        pt,
                        start=(n_pv == 1),
                        stop=(n_pv == npair),
                        perf_mode=mybir.MatmulPerfMode.DoubleRow,
                        skip_group_check=True,
                    )
                else:
                    vsrc = vbf if (PV_MODE == "fp8dr") else v8
                    for u in range(2):
                        nc.tensor.matmul(
                            pspv[0 : D + 2, :],
                            vsrc[:, 2 * jp + u, 0 : D + 2],
                            pt[:, u, :],
                            start=(n_pv == 1 and u == 0),
                            stop=(n_pv == npair and u == 1),
                            skip_group_check=True,
                        )
            # normalize on-chip: 1/sums lives at partition 64; broadcast
            # into this pspv tile's spare rows 64:128 with a K=1 matmul
            rzt = RZ.tile([65, 512], BF16, tag="rzt", name="rzt")
            with nc.allow_low_precision("softmax sums are O(100)"):
                nc.vector.reciprocal(out=rzt[64:65, :], in_=pspv[64:65, :])
            nc.tensor.matmul(
                pspv[64:128, :], ones64[64:65, :], rzt[64:65, :],
                start=True, stop=True, skip_group_check=True,
            )
            # DVE may read only one PSUM operand: stage the broadcast
            zbs = ON.tile([D, 512], BF16, tag="zbs", name="zbs")
            nc.vector.tensor_copy(out=zbs, in_=pspv[64:128, :])
            otn = ON.tile([D, 512], BF16, tag="otn", name="otn")
            nc.vector.tensor_mul(otn, pspv[0:D, :], zbs)
            # re-pair heads across partitions with identity matmuls into
            # this head's pspv tile: even head rows 0:64, odd rows 64:128
            pb = h // 2
            if h % 2 == 0:
                state["otn_even"] = otn
            if h % 2 == 1 or h == HG - 1:
                solo = h % 2 == 0
                nc.tensor.matmul(
                    pspv[0:64, :], perm_sb[0:64, 256:320], state["otn_even"],
                    start=True, stop=True, skip_group_check=True,
                )
                if not solo:
                    nc.tensor.matmul(
                        pspv[64:128, :], perm_sb[0:64, 256:320], otn,
                        start=True, stop=True, skip_group_check=True,
                    )
                    nc.vector.tensor_copy(out=otl[:, pb, :], in_=pspv[0:128, :])
                else:
                    nc.vector.tensor_copy(out=otl[0:64, pb, :], in_=pspv[0:64, :])

        def make_otl(c):
            odt = FP8 if c > 0 else BF16
            otl = OL.tile(
                [128, 4, 512], odt, tag="otl8" if c > 0 else "otlb", name="otl"
            )
            if c > 0:
                nc.vector.memset(otl[64:128, 3, :], 0.0)
            return otl

        def emit_attention_multi(chunk_offsets, steps_by_round, otls):
            """Interleave several chunks' head loops, each starting at its
            round offset, with explicit per-round extra work so the exp
            stream always has queued score tiles."""
            for c in chunk_offsets:
                otls[c] = make_otl(c)
            states = {c: {} for c in chunk_offsets}
            nrounds = max(
                [off + HG for off in chunk_offsets.values()]
                + [len(steps_by_round)]
            )
            for r in range(nrounds):
                for c, off in chunk_offsets.items():
                    hi = r - off
                    if 0 <= hi < HG:
                        h = [6, 0, 1, 2, 3, 4, 5][hi] if c == 0 else hi
                        att_head(c, h, otls[c], states[c])
                if r < len(steps_by_round):
                    for fn in steps_by_round[r]:
                        fn()

        def _slab(c, s, ps1=False):
            return lambda: proj_slab(c, s, ps1)

        def _v(c, tb, ps1=False):
            return lambda: proj_v(c, tb, ps1)

        # proj(0) essentials up front (borrowing the idle score buffers):
        # heads 0/1 need slab 0, head 6 and all scores need kTd (slab 3), V
        for st0 in (
            [_slab(0, 3), _slab(0, 0)]
            + [_v(0, tb) for tb in range(4)]
        ):
            st0()
        otls = {}
        p2 = proj_steps(2)
        emit_attention_multi(
            {0: 0, 1: 2},
            [
                [_slab(1, 3), _slab(0, 1), _v(1, 0), _v(1, 1)],
                [_slab(1, 0), _slab(0, 2), _v(1, 2), _v(1, 3)],
                [_slab(1, 1), _slab(1, 2)],
                p2[0:2], p2[2:4], p2[4:6], p2[6:8],
            ],
            otls,
        )
        p3 = proj_steps(3)
        op01 = [
            (lambda tb=tb, cc=cc: oproj_tb(cc, otls[cc], tb))
            for cc in (0, 1)
            for tb in range(4)
        ]
        emit_attention_multi(
            {2: 0},
            [
                p3[0:2], p3[2:4], p3[4:6], p3[6:8],
                op01[0:3], op01[3:6], op01[6:8],
            ],
            otls,
        )
        emit_attention_multi(
            {3: 0},
            [[(lambda tb=tb: oproj_tb(2, otls[2], tb))] for tb in range(4)],
            otls,
        )
        for tb in range(4):
            oproj_tb(3, otls[3], tb, use_ps1=True)

    nc.finalize()
    return nc


def _bf16(x):
    return np.ascontiguousarray(x).astype(_BF16)


def _fp8(x):
    return np.ascontiguousarray(x).astype(ml_dtypes.float8_e4m3fn)


def _prep_core(hidden, q_w, q_b, k_w, k_b, v_w, v_b, o_w, pos, b, g):
    hseq = hidden[S * b : S * (b + 1)]  # [S, HID]
    hT = np.ascontiguousarray(hseq.T).reshape(KBLK, 128, S)

    qg = q_w[:, NQ * g : NQ * (g + 1)]  # [HID, 448]
    kg = k_w[:, D * g : D * (g + 1)]  # [HID, 64]
    qk = np.concatenate([qg, kg], axis=1)  # [HID, 512]
    wqk_ = np.ascontiguousarray(qk).reshape(KBLK, 128, NQK)

    bq = np.concatenate([q_b[NQ * g : NQ * (g + 1)], k_b[D * g : D * (g + 1)]])
    bqk_ = np.ascontiguousarray(bq.reshape(NSLAB, 128).T)

    wv_ = np.ascontiguousarray(v_w[:, D * g : D * (g + 1)]).reshape(KBLK, 128, D)
    vb_ = np.concatenate(
        [v_b[D * g : D * (g + 1)], np.ones(2, np.float32)]
    ).reshape(1, D + 2)
    # o_proj weights: [448, HID] -> 4 partition blocks (last padded 64->128)
    ows = np.zeros((4, 128, HID), np.float32)
    ows.reshape(512, HID)[0:NQ] = o_w[NQ * g : NQ * (g + 1), :]

    p = pos[S * b : S * (b + 1)].astype(np.float32)
    inv_freq = 1.0 / (THETA ** (np.arange(0, D, 2, dtype=np.float32) / D))  # [32]
    ang = inv_freq[:, None] * p[None, :]  # [32, S]
    cos = np.ascontiguousarray(np.tile(np.cos(ang), (4, 1)))  # [128, S]
    sinpat_ = np.ascontiguousarray(np.tile(np.sin(ang), (4, 1)))  # [128, S]

    # perm[:, 0:128]: sign-folded rotate_half (block-diag per 64);
    # perm[:, 128:256]: duplicate rows 64:128 into both halves (for kTd);
    # perm[:, 256:320]: identity (rows 0:64) for the o_proj head re-pairing
    rot64 = np.zeros((64, 64), np.float32)
    for m in range(32):
        rot64[m + 32, m] = -1.0
        rot64[m, m + 32] = 1.0
    rblk = np.zeros((128, 128), np.float32)
    rblk[0:64, 0:64] = rot64
    rblk[64:128, 64:128] = rot64
    dup = np.zeros((128, 128), np.float32)
    for m in range(64):
        dup[64 + m, m] = 1.0
        dup[64 + m, 64 + m] = 1.0
    ident = np.zeros((128, 64), np.float32)
    ident[0:64, 0:64] = np.eye(64, dtype=np.float32)
    perm_ = np.ascontiguousarray(np.concatenate([rblk, dup, ident], axis=1))

    return {
        "hT": _fp8(hT),
        "hb0": _bf16(hT[:, :, 0:512]),
        "wqkb": _bf16(wqk_),
        "wvb": _bf16(wv_),
        "wqk": _fp8(wqk_),
        "wv": _fp8(wv_),
        "bqk": bqk_.astype(np.float32),
        "vb": _bf16(vb_),
        "ow": _fp8(ows),
        "owb": _bf16(ows),
        "cosf": _bf16(cos),
        "sinpat": _bf16(sinpat_),
        "perm": _bf16(perm_),
    }


def kernel(hidden_states, q_w, q_b, k_w, k_b, v_w, v_b, o_w, position_ids):
    hidden_states = np.asarray(hidden_states, dtype=np.float32)
    q_w = np.asarray(q_w, dtype=np.float32)
    q_b = np.asarray(q_b, dtype=np.float32)
    k_w = np.asarray(k_w, dtype=np.float32)
    k_b = np.asarray(k_b, dtype=np.float32)
    v_w = np.asarray(v_w, dtype=np.float32)
    v_b = np.asarray(v_b, dtype=np.float32)
    o_w = np.asarray(o_w, dtype=np.float32)
    position_ids = np.asarray(position_ids)

    if "nc" not in _CACHE:
        _CACHE["nc"] = _build()
    nc = _CACHE["nc"]

    in_maps = []
    for c in range(N_CORES):
        b, g = c // 2, c % 2
        in_maps.append(
            _prep_core(
                hidden_states, q_w, q_b, k_w, k_b, v_w, v_b, o_w, position_ids, b, g
            )
        )

    res = run_bass_kernel_spmd(nc, in_maps, core_ids=list(range(N_CORES)))
    parts = [np.asarray(r["out"], dtype=np.float32) for r in res.results]
    return np.concatenate(
        [parts[2 * b] + parts[2 * b + 1] for b in range(B)], axis=0
    ).astype(np.float32)


if __name__ == "__main__":
    rng = np.random.default_rng(0)
    T = B * S
    ins = {
        "hidden_states": rng.standard_normal((T, HID)).astype(np.float32),
        "q_w": (rng.standard_normal((HID, HID)) * 0.02).astype(np.float32),
        "q_b": (rng.standard_normal((HID,)) * 0.02).astype(np.float32),
        "k_w": (rng.standard_normal((HID, KV * D)) * 0.02).astype(np.float32),
        "k_b": (rng.standard_normal((KV * D,)) * 0.02).astype(np.float32),
        "v_w": (rng.standard_normal((HID, KV * D)) * 0.02).astype(np.float32),
        "v_b": (rng.standard_normal((KV * D,)) * 0.02).astype(np.float32),
        "o_w": (rng.standard_normal((HID, HID)) * 0.02).astype(np.float32),
        "position_ids": np.tile(np.arange(S, dtype=np.int32), B),
    }
    out = kernel(**ins)
    print("kernel output", out.shape, out.dtype, np.abs(out).max())


# revision 3
# speedup vs baseline: 1.0350x; 1.0350x over previous
"""Trainium2 Bass kernel for Qwen2-style causal self-attention (GQA + RoPE).

Geometry: B=4 seqs x S=2048 tokens, 14 Q heads / 2 KV heads, D=64, HID=896.
Sharding: 8 cores = 4 sequences x 2 head-groups (7 Q heads + 1 KV head each).
Each core computes its sequence's QKV projections (its head shard), RoPE,
causal attention, and a partial o_proj (448 input dims); the host sums the
two partials per sequence.

Design (driven by the TimelineSim cost model + real-HW numerics):
  - ScalarE is the bottleneck engine and does exp only (~139us); all
    scheduling aims to keep score PSUM tiles queued for it.
  - One continuous software pipeline: every chunk's head loop starts at a
    round offset and the projection / o_proj work is pinned to rounds that
    strictly precede its consumers, so each engine FIFO alternates between
    pipelines instead of head-of-line blocking on a phase.
  - fp8e4 DoubleRow (K=256/instruction, 0.5 cycles/row) for the QKV
    projections, PV, and o_proj of chunks 1-3; probs are quantized to fp8
    by the exp activation itself.  Chunk 0 stays bf16 end-to-end: a dot
    product's relative error does not average down with contraction size,
    and early tokens (few keys, large outputs) dominate the error budget.
  - causal masking on GPSIMD affine_select over the affected span only;
    the half of each second diagonal pair that is fully masked is never
    exp'd (GPSIMD zero-fills it instead).
  - softmax normalization per head stays on-chip: reciprocal of the [V|1]
    ones-column sums at partition 64, broadcast down via a K=1 matmul into
    the same PSUM tile's spare rows, staged once through SBUF (DVE may
    read only one PSUM operand), one multiply.
  - heads re-paired for o_proj (even head -> partitions 0:64, odd ->
    64:128) with identity matmuls into the same PSUM tile (both matmuls
    start=True: a start only clears its own region's has_written bits).
  - PSUM: scores 2x2 banks, pspv 2x1, proj 2x1 (each slab reuses one tile
    for the proj/rotate/dup outputs), o_proj shares the proj tag; the
    final o_proj borrows the then-idle score buffers.
  - PE p-state warmup matmuls run while the startup DMAs stream; DMAs are
    ordered/split by criticality (slab-0/3 weight columns, chunk-0 hidden
    pieces, chunk-0 cos/sin columns first).
"""

import numpy as np
from contextlib import ExitStack

import concourse.bacc as bacc
import concourse.bass as bass
import concourse.mybir as mybir
import concourse.tile as tile
from concourse.bass_utils import run_bass_kernel_spmd

import ml_dtypes

_BF16 = ml_dtypes.bfloat16

B, S = 4, 2048
H, KV, D = 14, 2, 64
HID = H * D  # 896
THETA = 1000000.0
G = 2  # tensor-parallel head groups
HG = H // G  # 7 q heads per group
NQ = HG * D  # 448
NQK = NQ + D  # 512 = q dims + k dims per group
KBLK = HID // 128  # 7 hid blocks
NSLAB = NQK // 128  # 4 slabs of the roped qk output
NTOK = S // 128  # 16 token blocks
NCHUNK = S // 512  # 4 token chunks
N_CORES = 8

F32 = mybir.dt.float32
BF16 = mybir.dt.bfloat16
FP8 = mybir.dt.float8e4
AF = mybir.ActivationFunctionType
ALU = mybir.AluOpType

PV_MODE = "fp8dr"  # "fp8dr" | "bf16"

_CACHE = {}


def _build():
    nc = bacc.Bacc("TRN2", target_bir_lowering=False, debug=False)

    hT = nc.dram_tensor("hT", [KBLK, 128, S], FP8, kind="ExternalInput")
    wqk = nc.dram_tensor("wqk", [KBLK, 128, NQK], FP8, kind="ExternalInput")
    wv = nc.dram_tensor("wv", [KBLK, 128, D], FP8, kind="ExternalInput")
    hb0 = nc.dram_tensor("hb0", [KBLK, 128, 512], BF16, kind="ExternalInput")
    wqkb = nc.dram_tensor("wqkb", [KBLK, 128, NQK], BF16, kind="ExternalInput")
    wvb = nc.dram_tensor("wvb", [KBLK, 128, D], BF16, kind="ExternalInput")
    bqk = nc.dram_tensor("bqk", [128, NSLAB], F32, kind="ExternalInput")
    vb = nc.dram_tensor("vb", [1, D + 2], BF16, kind="ExternalInput")
    ow = nc.dram_tensor("ow", [4, 128, HID], FP8, kind="ExternalInput")
    owb = nc.dram_tensor("owb", [4, 128, HID], BF16, kind="ExternalInput")
    cosf = nc.dram_tensor("cosf", [128, S], BF16, kind="ExternalInput")
    sinpat = nc.dram_tensor("sinpat", [128, S], BF16, kind="ExternalInput")
    perm = nc.dram_tensor("perm", [128, 320], BF16, kind="ExternalInput")
    out = nc.dram_tensor("out", [S, HID], BF16, kind="ExternalOutput")

    vdt = FP8 if PV_MODE == "fp8dr" else BF16

    with tile.TileContext(nc) as tc, ExitStack() as ctx:
        P = ctx.enter_context(tc.tile_pool(name="persist", bufs=1))
        RT = ctx.enter_context(tc.tile_pool(name="rt", bufs=2))
        PT = ctx.enter_context(tc.tile_pool(name="pt", bufs=4))
        RZ = ctx.enter_context(tc.tile_pool(name="rz", bufs=2))
        ON = ctx.enter_context(tc.tile_pool(name="on", bufs=4))
        OL = ctx.enter_context(tc.tile_pool(name="ol", bufs=3))
        OB = ctx.enter_context(tc.tile_pool(name="ob", bufs=2))
        PS1 = ctx.enter_context(tc.tile_pool(name="ps1", bufs=2, space="PSUM"))
        PS2 = ctx.enter_context(tc.tile_pool(name="ps2", bufs=2, space="PSUM"))
        PS3 = ctx.enter_context(tc.tile_pool(name="ps3", bufs=2, space="PSUM"))

        # ---- persistent tiles ----
        qk_sb = [P.tile([128, S], BF16, tag=f"qk{s}", name=f"qk{s}") for s in range(NSLAB)]
        kTd = P.tile([128, S], BF16, tag="kTd")
        cos_sb = P.tile([128, S], BF16, tag="cos")
        sin_sb = P.tile([128, S], BF16, tag="sin")
        bqk_sb = P.tile([128, NSLAB], F32, tag="bqk")
        wqk_sb = P.tile([128, KBLK, NQK], FP8, tag="wqk_sb")
        wv_sb = P.tile([128, KBLK, D], FP8, tag="wv_sb")
        ow_sb = P.tile([128, 4, HID], FP8, tag="ow_sb")
        owb_sb = P.tile([128, 4, HID], BF16, tag="owb_sb")
        vb_sb = P.tile([1, D + 2], BF16, tag="vb_sb")
        ones1 = P.tile([1, 128], BF16, tag="ones1")
        perm_sb = P.tile([128, 320], BF16, tag="perm_sb")
        v8 = P.tile([128, NTOK, 80], vdt, tag="v8")
        vbf = P.tile([128, 4, 80], BF16, tag="vbf")  # chunk-0 V in bf16
        h_sb = P.tile([128, KBLK, S], FP8, tag="h_sb")  # full hidden, resident
        hb0_sb = P.tile([128, KBLK, 512], BF16, tag="hb0_sb")  # chunk-0 hidden
        wqkb_sb = P.tile([128, KBLK, NQK], BF16, tag="wqkb_sb")
        wvb_sb = P.tile([128, KBLK, D], BF16, tag="wvb_sb")
        ones64 = P.tile([65, D], BF16, tag="ones64")
        ones512 = P.tile([1, 512], BF16, tag="ones512")

        # warm the PE p-state while the startup DMAs stream: the cost model
        # runs matmuls at half rate until ~3us of continuous PE activity
        nc.vector.memset(ones1, 1.0)
        nc.vector.memset(ones512, 1.0)
        for _w in range(8):
            wps = PS3.tile([128, 512], F32, tag="psA", name="warm")
            nc.tensor.matmul(wps, ones1, ones512, start=True, stop=True)

        # startup loads, most critical first: proj(0) slab 3 (K) needs the
        # slab-3 weight columns + chunk-0 hidden + bias + perm + cos/sin
        nc.sync.dma_start(
            out=wqkb_sb[:, :, 384:512],
            in_=wqkb[:, :, 384:512].rearrange("k p m -> p k m"),
        )
        nc.sync.dma_start(
            out=wqkb_sb[:, :, 0:128],
            in_=wqkb[:, :, 0:128].rearrange("k p m -> p k m"),
        )
        for k0, k1 in ((0, 2), (2, 4), (4, 6), (6, 7)):
            nc.sync.dma_start(
                out=hb0_sb[:, k0:k1, :],
                in_=hb0[k0:k1, :, :].rearrange("k p t -> p k t"),
            )
        nc.sync.dma_start(out=bqk_sb, in_=bqk[:, :])
        nc.sync.dma_start(out=perm_sb, in_=perm[:, :])
        nc.sync.dma_start(out=cos_sb[:, 0:512], in_=cosf[:, 0:512])
        nc.sync.dma_start(out=sin_sb[:, 0:512], in_=sinpat[:, 0:512])
        nc.sync.dma_start(out=cos_sb[:, 512:S], in_=cosf[:, 512:S])
        nc.sync.dma_start(out=sin_sb[:, 512:S], in_=sinpat[:, 512:S])
        nc.sync.dma_start(
            out=wqkb_sb[:, :, 128:384],
            in_=wqkb[:, :, 128:384].rearrange("k p m -> p k m"),
        )
        nc.sync.dma_start(out=wvb_sb, in_=wvb[:, :, :].rearrange("k p m -> p k m"))
        nc.sync.dma_start(out=wqk_sb, in_=wqk[:, :, :].rearrange("k p m -> p k m"))
        nc.sync.dma_start(out=wv_sb, in_=wv[:, :, :].rearrange("k p m -> p k m"))
        nc.sync.dma_start(out=vb_sb, in_=vb[:, :])
        for c in range(1, NCHUNK):
            nc.sync.dma_start(
                out=h_sb[:, :, 512 * c : 512 * c + 512],
                in_=hT[:, :, 512 * c : 512 * c + 512].rearrange("k p t -> p k t"),
            )
        nc.sync.dma_start(out=ow_sb, in_=ow[:, :, :].rearrange("b p m -> p b m"))
        nc.sync.dma_start(out=owb_sb, in_=owb[:, :, :].rearrange("b p m -> p b m"))
        nc.vector.memset(ones64, 1.0)

        def psa(use_ps1):
            if use_ps1:
                return PS1.tile([128, 1024], F32, tag="psS", name="psA")[:, 0:512]
            return PS3.tile([128, 512], F32, tag="psA", name="psA")

        def proj_slab(c, s, use_ps1=False):
            t0 = 512 * c
            ps = psa(use_ps1)
            if c == 0:
                # chunk-0 keys/values feed every later chunk and its outputs
                # are single-value-dominated: full bf16 projection
                for k in range(KBLK):
                    nc.tensor.matmul(
                        ps,
                        wqkb_sb[:, k, 128 * s : 128 * s + 128],
                        hb0_sb[:, k, :],
                        start=(k == 0),
                        stop=(k == KBLK - 1),
                    )
            else:
                h_t = h_sb[:, :, t0 : t0 + 512]
                for p in range(3):  # contraction pairs of hid blocks (fp8 DR)
                    nc.tensor.matmul(
                        ps,
                        wqk_sb[:, 2 * p : 2 * p + 2, 128 * s : 128 * s + 128],
                        h_t[:, 2 * p : 2 * p + 2, :],
                        start=(p == 0),
                        stop=False,
                        perf_mode=mybir.MatmulPerfMode.DoubleRow,
                        skip_group_check=True,
                    )
                nc.tensor.matmul(
                    ps,
                    wqk_sb[:, 6, 128 * s : 128 * s + 128],
                    h_t[:, 6, :],
                    start=False,
                    stop=True,
                    skip_group_check=True,
                )
            q = qk_sb[s][:, t0 : t0 + 512]
            nc.vector.tensor_scalar_add(q, ps, bqk_sb[:, s : s + 1])
            # rotate_half via sign-folded permutation matmul (cross-partition);
            # the slab's PSUM tile is reused for the rotate and dup outputs
            nc.tensor.matmul(ps, perm_sb[:, 0:128], q, start=True, stop=True)
            r = RT.tile([128, 512], BF16, tag="qkr", name="qkr")
            nc.vector.tensor_mul(r, ps, sin_sb[:, t0 : t0 + 512])
            nc.vector.tensor_mul(q, q, cos_sb[:, t0 : t0 + 512])
            nc.vector.tensor_add(q, q, r)
            if s == NSLAB - 1:
                # duplicated roped K^T (both partition halves)
                nc.tensor.matmul(ps, perm_sb[:, 128:256], q, start=True, stop=True)
                nc.vector.tensor_copy(out=kTd[:, t0 : t0 + 512], in_=ps)

        def proj_v(c, tb, use_ps1=False):
            t0 = 512 * c
            h_t = h_sb[:, :, t0 : t0 + 512]
            t = 4 * c + tb
            psv = psa(use_ps1)[:, 0 : D + 2]
            nc.tensor.matmul(
                psv, ones1, vb_sb, start=True, stop=False, skip_group_check=True
            )
            if c == 0:
                for k in range(KBLK):
                    nc.tensor.matmul(
                        psv[:, 0:D],
                        hb0_sb[:, k, 128 * tb : 128 * tb + 128],
                        wvb_sb[:, k, :],
                        start=False,
                        stop=(k == KBLK - 1),
                        skip_group_check=True,
                    )
            else:
                for p in range(3):
                    nc.tensor.matmul(
                        psv[:, 0:D],
                        h_t[:, 2 * p : 2 * p + 2, 128 * tb : 128 * tb + 128],
                        wv_sb[:, 2 * p : 2 * p + 2, :],
                        start=False,
                        stop=False,
                        perf_mode=mybir.MatmulPerfMode.DoubleRow,
                        skip_group_check=True,
                    )
                nc.tensor.matmul(
                    psv[:, 0:D],
                    h_t[:, 6, 128 * tb : 128 * tb + 128],
                    wv_sb[:, 6, :],
                    start=False,
                    stop=True,
                    skip_group_check=True,
                )
            nc.vector.tensor_copy(out=v8[:, t, 0 : D + 2], in_=psv)
            if c == 0:
                nc.vector.tensor_copy(out=vbf[:, t, 0 : D + 2], in_=psv)

        def proj_steps(c, use_ps1=False):
            steps = [lambda s=s: proj_slab(c, s, use_ps1) for s in (3, 0, 1, 2)]
            steps += [lambda tb=tb: proj_v(c, tb, use_ps1) for tb in range(4)]
            return steps

        def oproj_tb(c, otl, tb, use_ps1=False):
            t = 4 * c + tb
            ob = OB.tile([128, HID], BF16, tag="ob", name="ob")
            for n0, n1 in ((0, 512), (512, HID)):
                if use_ps1:
                    po = PS1.tile([128, 1024], F32, tag="psS", name="po")[:, 0:512]
                else:
                    po = PS3.tile([128, 512], F32, tag="psA", name="po")
                if c > 0:
                    # fp8 DR: contract head-pair blocks two at a time (K=256)
                    for i in range(2):
                        nc.tensor.matmul(
                            po[:, 0 : n1 - n0],
                            otl[:, 2 * i : 2 * i + 2, 128 * tb : 128 * tb + 128],
                            ow_sb[:, 2 * i : 2 * i + 2, n0:n1],
                            start=(i == 0),
                            stop=(i == 1),
                            perf_mode=mybir.MatmulPerfMode.DoubleRow,
                            skip_group_check=True,
                        )
                else:
                    for pb in range(4):
                        p_n = 128 if pb < 3 else 64
                        nc.tensor.matmul(
                            po[:, 0 : n1 - n0],
                            otl[0:p_n, pb, 128 * tb : 128 * tb + 128],
                            owb_sb[0:p_n, pb, n0:n1],
                            start=(pb == 0),
                            stop=(pb == 3),
                        )
                if use_ps1 and n0 == 0:
                    nc.scalar.copy(out=ob[:, n0:n1], in_=po[:, 0 : n1 - n0])
                else:
                    nc.vector.tensor_copy(
                        out=ob[:, n0:n1], in_=po[:, 0 : n1 - n0]
                    )
            nc.sync.dma_start(out=out[128 * t : 128 * t + 128, :], in_=ob)

        def att_head(c, h, otl, state):
            t0 = 512 * c
            npair = 2 * c + 2
            use_dr = PV_MODE == "fp8dr" and c > 0
            ptdt = vdt if use_dr else BF16
            # diagonal pairs first so gpsimd masking overlaps later blocks
            jp_order = [2 * c, 2 * c + 1] + list(range(0, 2 * c))
            slab = h // 2
            row = 64 * (h % 2)
            q_ap = qk_sb[slab][row : row + 64, t0 : t0 + 512]
            pspv = PS2.tile([128, 512], F32, tag="pspv", name="pspv")
            n_pv = 0
            for jp in jp_order:
                pss = PS1.tile([128, 1024], F32, tag="psS", name="psS")
                diag_b = jp == 2 * c + 1  # blocks 4c+2/4c+3: half the columns
                # of this pair are fully above the diagonal; don't exp them
                for u in range(2):
                    j = 2 * jp + u
                    nc.tensor.matmul(
                        pss[:, 512 * u : 512 * u + 512],
                        kTd[row : row + 64, 128 * j : 128 * j + 128],
                        q_ap,
                        start=True,
                        stop=True,
                    )
                pt = PT.tile(
                    [128, 2, 512], ptdt,
                    tag="pt" if use_dr else "ptb",
                    name="pt",
                )
                if diag_b:
                    nc.gpsimd.memset(pt[:, 0, 0:256], 0.0)
                    nc.gpsimd.memset(pt[:, 1, 0:384], 0.0)
                    nc.scalar.activation(
                        out=pt[:, 0, 256:512], in_=pss[:, 256:512],
                        func=AF.Exp, scale=0.125,
                    )
                    nc.scalar.activation(
                        out=pt[:, 1, 384:512], in_=pss[:, 896:1024],
                        func=AF.Exp, scale=0.125,
                    )
                    for u in range(2):
                        off = 256 + 128 * u
                        nc.gpsimd.affine_select(
                            out=pt[:, u, off : off + 128],
                            in_=pt[:, u, off : off + 128],
                            compare_op=ALU.is_ge,
                            fill=0.0,
                            base=0,
                            channel_multiplier=-1,
                            pattern=[[1, 128]],
                        )
                else:
                    nc.scalar.activation(
                        out=pt.rearrange("p a b -> p (a b)"), in_=pss,
                        func=AF.Exp, scale=0.125,
                    )
                    for u in range(2):
                        j = 2 * jp + u
                        if j >= 4 * c:  # diagonal block: zero out k > q
                            span = 128 * (j - 4 * c) + 128
                            nc.gpsimd.affine_select(
                                out=pt[:, u, 0:span],
                                in_=pt[:, u, 0:span],
                                compare_op=ALU.is_ge,
                                fill=0.0,
                                base=t0 - 128 * j,
                                channel_multiplier=-1,
                                pattern=[[1, span]],
                            )
                n_pv += 1
                if use_dr:
                    nc.tensor.matmul(
                        pspv[0 : D + 2, :],
                        v8[:, 2 * jp : 2 * jp + 2, 0 : D + 2],
                        pt,
                        start=(n_pv == 1),
                        stop=(n_pv == npair),
                        perf_mode=mybir.MatmulPerfMode.DoubleRow,
                        skip_group_check=True,
                    )
                else:
                    vsrc = vbf if (PV_MODE == "fp8dr") else v8
                    for u in range(2):
                        nc.tensor.matmul(
                            pspv[0 : D + 2, :],
                            vsrc[:, 2 * jp + u, 0 : D + 2],
                            pt[:, u, :],
                            start=(n_pv == 1 and u == 0),
                            stop=(n_pv == npair and u == 1),
                            skip_group_check=True,
                        )
            # normalize on-chip: 1/sums lives at partition 64; broadcast
            # into this pspv tile's spare rows 64:128 with a K=1 matmul
            rzt = RZ.tile([65, 512], BF16, tag="rzt", name="rzt")
            with nc.allow_low_precision("softmax sums are O(100)"):
                nc.vector.reciprocal(out=rzt[64:65, :], in_=pspv[64:65, :])
            nc.tensor.matmul(
                pspv[64:128, :], ones64[64:65, :], rzt[64:65, :],
                start=True, stop=True, skip_group_check=True,
            )
            # DVE may read only one PSUM operand: stage the broadcast
            zbs = ON.tile([D, 512], BF16, tag="zbs", name="zbs")
            nc.vector.tensor_copy(out=zbs, in_=pspv[64:128, :])
            otn = ON.tile([D, 512], BF16, tag="otn", name="otn")
            nc.vector.tensor_mul(otn, pspv[0:D, :], zbs)
            # re-pair heads across partitions with identity matmuls into
            # this head's pspv tile: even head rows 0:64, odd rows 64:128
            pb = h // 2
            if h % 2 == 0:
                state["otn_even"] = otn
            if h % 2 == 1 or h == HG - 1:
                solo = h % 2 == 0
                nc.tensor.matmul(
                    pspv[0:64, :], perm_sb[0:64, 256:320], state["otn_even"],
                    start=True, stop=True, skip_group_check=True,
                )
                if not solo:
                    nc.tensor.matmul(
                        pspv[64:128, :], perm_sb[0:64, 256:320], otn,
                        start=True, stop=True, skip_group_check=True,
                    )
                    nc.vector.tensor_copy(out=otl[:, pb, :], in_=pspv[0:128, :])
                else:
                    nc.vector.tensor_copy(out=otl[0:64, pb, :], in_=pspv[0:64, :])

        def make_otl(c):
            odt = FP8 if c > 0 else BF16
            otl = OL.tile(
                [128, 4, 512], odt, tag="otl8" if c > 0 else "otlb", name="otl"
            )
            if c > 0:
                nc.vector.memset(otl[64:128, 3, :], 0.0)
            return otl

        def emit_attention_multi(chunk_offsets, steps_by_round, otls):
            """Interleave several chunks' head loops, each starting at its
            round offset, with explicit per-round extra work so the exp
            stream always has queued score tiles."""
            for c in chunk_offsets:
                otls[c] = make_otl(c)
            states = {c: {} for c in chunk_offsets}
            nrounds = max(
                [off + HG for off in chunk_offsets.values()]
                + [len(steps_by_round)]
            )
            for r in range(nrounds):
                for c, off in chunk_offsets.items():
                    hi = r - off
                    if 0 <= hi < HG:
                        h = [6, 0, 1, 2, 3, 4, 5][hi] if c == 0 else hi
                        att_head(c, h, otls[c], states[c])
                if r < len(steps_by_round):
                    for fn in steps_by_round[r]:
                        fn()

        def _slab(c, s, ps1=False):
            return lambda: proj_slab(c, s, ps1)

        def _v(c, tb, ps1=False):
            return lambda: proj_v(c, tb, ps1)

        # proj(0) essentials up front (borrowing the idle score buffers):
        # heads 0/1 need slab 0, head 6 and all scores need kTd (slab 3), V
        for st0 in (
            [_slab(0, 3), _slab(0, 0)]
            + [_v(0, tb) for tb in range(4)]
        ):
            st0()
        otls = {}
        p2 = proj_steps(2)
        p3 = proj_steps(3)

        def _op(cc, tb):
            return lambda: oproj_tb(cc, otls[cc], tb)

        # one continuous pipeline: chunk c's heads start at its round offset,
        # with projections/o_proj token-blocks pinned to rounds that strictly
        # precede their consumers
        emit_attention_multi(
            {0: 0, 1: 2, 2: 6, 3: 11},
            [
                [_slab(1, 3), _slab(0, 1), _v(1, 0), _v(1, 1)],
                [_slab(1, 0), _slab(0, 2), _v(1, 2), _v(1, 3)],
                [_slab(1, 1), _slab(1, 2)],
                [p2[0], p2[1], p2[4], p2[5]],
                [p2[2], p2[3], p2[6], p2[7]],
                [],
                [],
                [p3[0], p3[1], p3[4], p3[5]],
                [p3[2], p3[3], p3[6], p3[7]],
                [_op(0, 0), _op(0, 1)],
                [_op(0, 2), _op(0, 3)],
                [_op(1, 0), _op(1, 1)],
                [_op(1, 2), _op(1, 3)],
                [_op(2, 0), _op(2, 1)],
                [_op(2, 2), _op(2, 3)],
            ],
            otls,
        )
        for tb in range(4):
            oproj_tb(3, otls[3], tb, use_ps1=True)

    nc.finalize()
    return nc


def _bf16(x):
    return np.ascontiguousarray(x).astype(_BF16)


def _fp8(x):
    return np.ascontiguousarray(x).astype(ml_dtypes.float8_e4m3fn)


def _prep_core(hidden, q_w, q_b, k_w, k_b, v_w, v_b, o_w, pos, b, g):
    hseq = hidden[S * b : S * (b + 1)]  # [S, HID]
    hT = np.ascontiguousarray(hseq.T).reshape(KBLK, 128, S)

    qg = q_w[:, NQ * g : NQ * (g + 1)]  # [HID, 448]
    kg = k_w[:, D * g : D * (g + 1)]  # [HID, 64]
    qk = np.concatenate([qg, kg], axis=1)  # [HID, 512]
    wqk_ = np.ascontiguousarray(qk).reshape(KBLK, 128, NQK)

    bq = np.concatenate([q_b[NQ * g : NQ * (g + 1)], k_b[D * g : D * (g + 1)]])
    bqk_ = np.ascontiguousarray(bq.reshape(NSLAB, 128).T)

    wv_ = np.ascontiguousarray(v_w[:, D * g : D * (g + 1)]).reshape(KBLK, 128, D)
    vb_ = np.concatenate(
        [v_b[D * g : D * (g + 1)], np.ones(2, np.float32)]
    ).reshape(1, D + 2)
    # o_proj weights: [448, HID] -> 4 partition blocks (last padded 64->128)
    ows = np.zeros((4, 128, HID), np.float32)
    ows.reshape(512, HID)[0:NQ] = o_w[NQ * g : NQ * (g + 1), :]

    p = pos[S * b : S * (b + 1)].astype(np.float32)
    inv_freq = 1.0 / (THETA ** (np.arange(0, D, 2, dtype=np.float32) / D))  # [32]
    ang = inv_freq[:, None] * p[None, :]  # [32, S]
    cos = np.ascontiguousarray(np.tile(np.cos(ang), (4, 1)))  # [128, S]
    sinpat_ = np.ascontiguousarray(np.tile(np.sin(ang), (4, 1)))  # [128, S]

    # perm[:, 0:128]: sign-folded rotate_half (block-diag per 64);
    # perm[:, 128:256]: duplicate rows 64:128 into both halves (for kTd);
    # perm[:, 256:320]: identity (rows 0:64) for the o_proj head re-pairing
    rot64 = np.zeros((64, 64), np.float32)
    for m in range(32):
        rot64[m + 32, m] = -1.0
        rot64[m, m + 32] = 1.0
    rblk = np.zeros((128, 128), np.float32)
    rblk[0:64, 0:64] = rot64
    rblk[64:128, 64:128] = rot64
    dup = np.zeros((128, 128), np.float32)
    for m in range(64):
        dup[64 + m, m] = 1.0
        dup[64 + m, 64 + m] = 1.0
    ident = np.zeros((128, 64), np.float32)
    ident[0:64, 0:64] = np.eye(64, dtype=np.float32)
    perm_ = np.ascontiguousarray(np.concatenate([rblk, dup, ident], axis=1))

    return {
        "hT": _fp8(hT),
        "hb0": _bf16(hT[:, :, 0:512]),
        "wqkb": _bf16(wqk_),
        "wvb": _bf16(wv_),
        "wqk": _fp8(wqk_),
        "wv": _fp8(wv_),
        "bqk": bqk_.astype(np.float32),
        "vb": _bf16(vb_),
        "ow": _fp8(ows),
        "owb": _bf16(ows),
        "cosf": _bf16(cos),
        "sinpat": _bf16(sinpat_),
        "perm": _bf16(perm_),
    }


def kernel(hidden_states, q_w, q_b, k_w, k_b, v_w, v_b, o_w, position_ids):
    hidden_states = np.asarray(hidden_states, dtype=np.float32)
    q_w = np.asarray(q_w, dtype=np.float32)
    q_b = np.asarray(q_b, dtype=np.float32)
    k_w = np.asarray(k_w, dtype=np.float32)
    k_b = np.asarray(k_b, dtype=np.float32)
    v_w = np.asarray(v_w, dtype=np.float32)
    v_b = np.asarray(v_b, dtype=np.float32)
    o_w = np.asarray(o_w, dtype=np.float32)
    position_ids = np.asarray(position_ids)

    if "nc" not in _CACHE:
        _CACHE["nc"] = _build()
    nc = _CACHE["nc"]

    in_maps = []
    for c in range(N_CORES):
        b, g = c // 2, c % 2
        in_maps.append(
            _prep_core(
                hidden_states, q_w, q_b, k_w, k_b, v_w, v_b, o_w, position_ids, b, g
            )
        )

    res = run_bass_kernel_spmd(nc, in_maps, core_ids=list(range(N_CORES)))
    parts = [np.asarray(r["out"], dtype=np.float32) for r in res.results]
    return np.concatenate(
        [parts[2 * b] + parts[2 * b + 1] for b in range(B)], axis=0
    ).astype(np.float32)


if __name__ == "__main__":
    rng = np.random.default_rng(0)
    T = B * S
    ins = {
        "hidden_states": rng.standard_normal((T, HID)).astype(np.float32),
        "q_w": (rng.standard_normal((HID, HID)) * 0.02).astype(np.float32),
        "q_b": (rng.standard_normal((HID,)) * 0.02).astype(np.float32),
        "k_w": (rng.standard_normal((HID, KV * D)) * 0.02).astype(np.float32),
        "k_b": (rng.standard_normal((KV * D,)) * 0.02).astype(np.float32),
        "v_w": (rng.standard_normal((HID, KV * D)) * 0.02).astype(np.float32),
        "v_b": (rng.standard_normal((KV * D,)) * 0.02).astype(np.float32),
        "o_w": (rng.standard_normal((HID, HID)) * 0.02).astype(np.float32),
        "position_ids": np.tile(np.arange(S, dtype=np.int32), B),
    }
    out = kernel(**ins)
    print("kernel output", out.shape, out.dtype, np.abs(out).max())


# revision 4
# speedup vs baseline: 1.0352x; 1.0002x over previous
"""Trainium2 Bass kernel for Qwen2-style causal self-attention (GQA + RoPE).

Geometry: B=4 seqs x S=2048 tokens, 14 Q heads / 2 KV heads, D=64, HID=896.
Sharding: 8 cores = 4 sequences x 2 head-groups (7 Q heads + 1 KV head each).
Each core computes its sequence's QKV projections (its head shard), RoPE,
causal attention, and a partial o_proj (448 input dims); the host sums the
two partials per sequence.

Design (driven by the TimelineSim cost model + real-HW numerics):
  - ScalarE is the bottleneck engine and does exp only (~139us); all
    scheduling aims to keep score PSUM tiles queued for it.
  - One continuous software pipeline: every chunk's head loop starts at a
    round offset and the projection / o_proj work is pinned to rounds that
    strictly precede its consumers, so each engine FIFO alternates between
    pipelines instead of head-of-line blocking on a phase.
  - fp8e4 DoubleRow (K=256/instruction, 0.5 cycles/row) for the QKV
    projections, PV, and o_proj of chunks 1-3; probs are quantized to fp8
    by the exp activation itself.  Chunk 0 stays bf16 end-to-end: a dot
    product's relative error does not average down with contraction size,
    and early tokens (few keys, large outputs) dominate the error budget.
  - causal masking on GPSIMD affine_select over the affected span only;
    the half of each second diagonal pair that is fully masked is never
    exp'd (GPSIMD zero-fills it instead).
  - softmax normalization per head stays on-chip: reciprocal of the [V|1]
    ones-column sums at partition 64, broadcast down via a K=1 matmul into
    the same PSUM tile's spare rows, staged once through SBUF (DVE may
    read only one PSUM operand), one multiply.
  - heads re-paired for o_proj (even head -> partitions 0:64, odd ->
    64:128) with identity matmuls into the same PSUM tile (both matmuls
    start=True: a start only clears its own region's has_written bits).
  - PSUM: scores 2x2 banks, pspv 2x1, proj 2x1 (each slab reuses one tile
    for the proj/rotate/dup outputs), o_proj shares the proj tag; the
    final o_proj borrows the then-idle score buffers.
  - PE p-state warmup matmuls run while the startup DMAs stream; DMAs are
    ordered/split by criticality (slab-0/3 weight columns, chunk-0 hidden
    pieces, chunk-0 cos/sin columns first).
"""

import numpy as np
from contextlib import ExitStack

import concourse.bacc as bacc
import concourse.bass as bass
import concourse.mybir as mybir
import concourse.tile as tile
from concourse.bass_utils import run_bass_kernel_spmd

import ml_dtypes

_BF16 = ml_dtypes.bfloat16

B, S = 4, 2048
H, KV, D = 14, 2, 64
HID = H * D  # 896
THETA = 1000000.0
G = 2  # tensor-parallel head groups
HG = H // G  # 7 q heads per group
NQ = HG * D  # 448
NQK = NQ + D  # 512 = q dims + k dims per group
KBLK = HID // 128  # 7 hid blocks
NSLAB = NQK // 128  # 4 slabs of the roped qk output
NTOK = S // 128  # 16 token blocks
NCHUNK = S // 512  # 4 token chunks
N_CORES = 8

F32 = mybir.dt.float32
BF16 = mybir.dt.bfloat16
FP8 = mybir.dt.float8e4
AF = mybir.ActivationFunctionType
ALU = mybir.AluOpType

PV_MODE = "fp8dr"  # "fp8dr" | "bf16"

_CACHE = {}


def _build():
    nc = bacc.Bacc("TRN2", target_bir_lowering=False, debug=False)

    hT = nc.dram_tensor("hT", [KBLK, 128, S], FP8, kind="ExternalInput")
    wqk = nc.dram_tensor("wqk", [KBLK, 128, NQK], FP8, kind="ExternalInput")
    wv = nc.dram_tensor("wv", [KBLK, 128, D], FP8, kind="ExternalInput")
    hb0 = nc.dram_tensor("hb0", [KBLK, 128, 512], BF16, kind="ExternalInput")
    wqkb = nc.dram_tensor("wqkb", [KBLK, 128, NQK], BF16, kind="ExternalInput")
    wvb = nc.dram_tensor("wvb", [KBLK, 128, D], BF16, kind="ExternalInput")
    bqk = nc.dram_tensor("bqk", [128, NSLAB], F32, kind="ExternalInput")
    vb = nc.dram_tensor("vb", [1, D + 2], BF16, kind="ExternalInput")
    ow = nc.dram_tensor("ow", [4, 128, HID], FP8, kind="ExternalInput")
    owb = nc.dram_tensor("owb", [4, 128, HID], BF16, kind="ExternalInput")
    cosf = nc.dram_tensor("cosf", [128, S], BF16, kind="ExternalInput")
    sinpat = nc.dram_tensor("sinpat", [128, S], BF16, kind="ExternalInput")
    perm = nc.dram_tensor("perm", [128, 320], BF16, kind="ExternalInput")
    out = nc.dram_tensor("out", [S, HID], BF16, kind="ExternalOutput")

    vdt = FP8 if PV_MODE == "fp8dr" else BF16

    with tile.TileContext(nc) as tc, ExitStack() as ctx:
        P = ctx.enter_context(tc.tile_pool(name="persist", bufs=1))
        RT = ctx.enter_context(tc.tile_pool(name="rt", bufs=4))
        PT = ctx.enter_context(tc.tile_pool(name="pt", bufs=12))
        RZ = ctx.enter_context(tc.tile_pool(name="rz", bufs=8))
        ON = ctx.enter_context(tc.tile_pool(name="on", bufs=12))
        OL = ctx.enter_context(tc.tile_pool(name="ol", bufs=4))
        OB = ctx.enter_context(tc.tile_pool(name="ob", bufs=4))
        PS1 = ctx.enter_context(tc.tile_pool(name="ps1", bufs=2, space="PSUM"))
        PS2 = ctx.enter_context(tc.tile_pool(name="ps2", bufs=2, space="PSUM"))
        PS3 = ctx.enter_context(tc.tile_pool(name="ps3", bufs=2, space="PSUM"))

        # ---- persistent tiles ----
        qk_sb = [P.tile([128, S], BF16, tag=f"qk{s}", name=f"qk{s}") for s in range(NSLAB)]
        kTd = P.tile([128, S], BF16, tag="kTd")
        cos_sb = P.tile([128, S], BF16, tag="cos")
        sin_sb = P.tile([128, S], BF16, tag="sin")
        bqk_sb = P.tile([128, NSLAB], F32, tag="bqk")
        wqk_sb = P.tile([128, KBLK, NQK], FP8, tag="wqk_sb")
        wv_sb = P.tile([128, KBLK, D], FP8, tag="wv_sb")
        ow_sb = P.tile([128, 4, HID], FP8, tag="ow_sb")
        owb_sb = P.tile([128, 4, HID], BF16, tag="owb_sb")
        vb_sb = P.tile([1, D + 2], BF16, tag="vb_sb")
        ones1 = P.tile([1, 128], BF16, tag="ones1")
        perm_sb = P.tile([128, 320], BF16, tag="perm_sb")
        v8 = P.tile([128, NTOK, 80], vdt, tag="v8")
        vbf = P.tile([128, 4, 80], BF16, tag="vbf")  # chunk-0 V in bf16
        h_sb = P.tile([128, KBLK, S], FP8, tag="h_sb")  # full hidden, resident
        hb0_sb = P.tile([128, KBLK, 512], BF16, tag="hb0_sb")  # chunk-0 hidden
        wqkb_sb = P.tile([128, KBLK, NQK], BF16, tag="wqkb_sb")
        wvb_sb = P.tile([128, KBLK, D], BF16, tag="wvb_sb")
        ones64 = P.tile([65, D], BF16, tag="ones64")
        ones512 = P.tile([1, 512], BF16, tag="ones512")

        # warm the PE p-state while the startup DMAs stream: the cost model
        # runs matmuls at half rate until ~3us of continuous PE activity
        nc.vector.memset(ones1, 1.0)
        nc.vector.memset(ones512, 1.0)
        for _w in range(8):
            wps = PS3.tile([128, 512], F32, tag="psA", name="warm")
            nc.tensor.matmul(wps, ones1, ones512, start=True, stop=True)

        # startup loads, most critical first: proj(0) slab 3 (K) needs the
        # slab-3 weight columns + chunk-0 hidden + bias + perm + cos/sin
        nc.sync.dma_start(
            out=wqkb_sb[:, :, 384:512],
            in_=wqkb[:, :, 384:512].rearrange("k p m -> p k m"),
        )
        nc.sync.dma_start(
            out=wqkb_sb[:, :, 0:128],
            in_=wqkb[:, :, 0:128].rearrange("k p m -> p k m"),
        )
        for k0, k1 in ((0, 2), (2, 4), (4, 6), (6, 7)):
            nc.sync.dma_start(
                out=hb0_sb[:, k0:k1, :],
                in_=hb0[k0:k1, :, :].rearrange("k p t -> p k t"),
            )
        nc.sync.dma_start(out=bqk_sb, in_=bqk[:, :])
        nc.sync.dma_start(out=perm_sb, in_=perm[:, :])
        nc.sync.dma_start(out=cos_sb[:, 0:512], in_=cosf[:, 0:512])
        nc.sync.dma_start(out=sin_sb[:, 0:512], in_=sinpat[:, 0:512])
        nc.sync.dma_start(out=cos_sb[:, 512:S], in_=cosf[:, 512:S])
        nc.sync.dma_start(out=sin_sb[:, 512:S], in_=sinpat[:, 512:S])
        nc.sync.dma_start(
            out=wqkb_sb[:, :, 128:384],
            in_=wqkb[:, :, 128:384].rearrange("k p m -> p k m"),
        )
        nc.sync.dma_start(out=wvb_sb, in_=wvb[:, :, :].rearrange("k p m -> p k m"))
        nc.sync.dma_start(out=wqk_sb, in_=wqk[:, :, :].rearrange("k p m -> p k m"))
        nc.sync.dma_start(out=wv_sb, in_=wv[:, :, :].rearrange("k p m -> p k m"))
        nc.sync.dma_start(out=vb_sb, in_=vb[:, :])
        for c in range(1, NCHUNK):
            nc.sync.dma_start(
                out=h_sb[:, :, 512 * c : 512 * c + 512],
                in_=hT[:, :, 512 * c : 512 * c + 512].rearrange("k p t -> p k t"),
            )
        nc.sync.dma_start(out=ow_sb, in_=ow[:, :, :].rearrange("b p m -> p b m"))
        nc.sync.dma_start(out=owb_sb, in_=owb[:, :, :].rearrange("b p m -> p b m"))
        nc.vector.memset(ones64, 1.0)

        def psa(use_ps1):
            if use_ps1:
                return PS1.tile([128, 1024], F32, tag="psS", name="psA")[:, 0:512]
            return PS3.tile([128, 512], F32, tag="psA", name="psA")

        def proj_slab(c, s, use_ps1=False):
            t0 = 512 * c
            ps = psa(use_ps1)
            if c == 0:
                # chunk-0 keys/values feed every later chunk and its outputs
                # are single-value-dominated: full bf16 projection
                for k in range(KBLK):
                    nc.tensor.matmul(
                        ps,
                        wqkb_sb[:, k, 128 * s : 128 * s + 128],
                        hb0_sb[:, k, :],
                        start=(k == 0),
                        stop=(k == KBLK - 1),
                    )
            else:
                h_t = h_sb[:, :, t0 : t0 + 512]
                for p in range(3):  # contraction pairs of hid blocks (fp8 DR)
                    nc.tensor.matmul(
                        ps,
                        wqk_sb[:, 2 * p : 2 * p + 2, 128 * s : 128 * s + 128],
                        h_t[:, 2 * p : 2 * p + 2, :],
                        start=(p == 0),
                        stop=False,
                        perf_mode=mybir.MatmulPerfMode.DoubleRow,
                        skip_group_check=True,
                    )
                nc.tensor.matmul(
                    ps,
                    wqk_sb[:, 6, 128 * s : 128 * s + 128],
                    h_t[:, 6, :],
                    start=False,
                    stop=True,
                    skip_group_check=True,
                )
            q = qk_sb[s][:, t0 : t0 + 512]
            nc.vector.tensor_scalar_add(q, ps, bqk_sb[:, s : s + 1])
            # rotate_half via sign-folded permutation matmul (cross-partition);
            # the slab's PSUM tile is reused for the rotate and dup outputs
            nc.tensor.matmul(ps, perm_sb[:, 0:128], q, start=True, stop=True)
            r = RT.tile([128, 512], BF16, tag="qkr", name="qkr")
            nc.vector.tensor_mul(r, ps, sin_sb[:, t0 : t0 + 512])
            nc.vector.tensor_mul(q, q, cos_sb[:, t0 : t0 + 512])
            nc.vector.tensor_add(q, q, r)
            if s == NSLAB - 1:
                # duplicated roped K^T (both partition halves)
                nc.tensor.matmul(ps, perm_sb[:, 128:256], q, start=True, stop=True)
                nc.vector.tensor_copy(out=kTd[:, t0 : t0 + 512], in_=ps)

        def proj_v(c, tb, use_ps1=False):
            t0 = 512 * c
            h_t = h_sb[:, :, t0 : t0 + 512]
            t = 4 * c + tb
            psv = psa(use_ps1)[:, 0 : D + 2]
            nc.tensor.matmul(
                psv, ones1, vb_sb, start=True, stop=False, skip_group_check=True
            )
            if c == 0:
                for k in range(KBLK):
                    nc.tensor.matmul(
                        psv[:, 0:D],
                        hb0_sb[:, k, 128 * tb : 128 * tb + 128],
                        wvb_sb[:, k, :],
                        start=False,
                        stop=(k == KBLK - 1),
                        skip_group_check=True,
                    )
            else:
                for p in range(3):
                    nc.tensor.matmul(
                        psv[:, 0:D],
                        h_t[:, 2 * p : 2 * p + 2, 128 * tb : 128 * tb + 128],
                        wv_sb[:, 2 * p : 2 * p + 2, :],
                        start=False,
                        stop=False,
                        perf_mode=mybir.MatmulPerfMode.DoubleRow,
                        skip_group_check=True,
                    )
                nc.tensor.matmul(
                    psv[:, 0:D],
                    h_t[:, 6, 128 * tb : 128 * tb + 128],
                    wv_sb[:, 6, :],
                    start=False,
                    stop=True,
                    skip_group_check=True,
                )
            nc.vector.tensor_copy(out=v8[:, t, 0 : D + 2], in_=psv)
            if c == 0:
                nc.vector.tensor_copy(out=vbf[:, t, 0 : D + 2], in_=psv)

        def proj_steps(c, use_ps1=False):
            steps = [lambda s=s: proj_slab(c, s, use_ps1) for s in (3, 0, 1, 2)]
            steps += [lambda tb=tb: proj_v(c, tb, use_ps1) for tb in range(4)]
            return steps

        def oproj_tb(c, otl, tb, use_ps1=False):
            t = 4 * c + tb
            ob = OB.tile([128, HID], BF16, tag="ob", name="ob")
            for n0, n1 in ((0, 512), (512, HID)):
                if use_ps1:
                    po = PS1.tile([128, 1024], F32, tag="psS", name="po")[:, 0:512]
                else:
                    po = PS3.tile([128, 512], F32, tag="psA", name="po")
                if c > 0:
                    # fp8 DR: contract head-pair blocks two at a time (K=256)
                    for i in range(2):
                        nc.tensor.matmul(
                            po[:, 0 : n1 - n0],
                            otl[:, 2 * i : 2 * i + 2, 128 * tb : 128 * tb + 128],
                            ow_sb[:, 2 * i : 2 * i + 2, n0:n1],
                            start=(i == 0),
                            stop=(i == 1),
                            perf_mode=mybir.MatmulPerfMode.DoubleRow,
                            skip_group_check=True,
                        )
                else:
                    for pb in range(4):
                        p_n = 128 if pb < 3 else 64
                        nc.tensor.matmul(
                            po[:, 0 : n1 - n0],
                            otl[0:p_n, pb, 128 * tb : 128 * tb + 128],
                            owb_sb[0:p_n, pb, n0:n1],
                            start=(pb == 0),
                            stop=(pb == 3),
                        )
                if use_ps1 and n0 == 0:
                    nc.scalar.copy(out=ob[:, n0:n1], in_=po[:, 0 : n1 - n0])
                else:
                    nc.vector.tensor_copy(
                        out=ob[:, n0:n1], in_=po[:, 0 : n1 - n0]
                    )
            nc.sync.dma_start(out=out[128 * t : 128 * t + 128, :], in_=ob)

        def att_head(c, h, otl, state):
            t0 = 512 * c
            npair = 2 * c + 2
            use_dr = PV_MODE == "fp8dr" and c > 0
            ptdt = vdt if use_dr else BF16
            # diagonal pairs first so gpsimd masking overlaps later blocks
            jp_order = [2 * c, 2 * c + 1] + list(range(0, 2 * c))
            slab = h // 2
            row = 64 * (h % 2)
            q_ap = qk_sb[slab][row : row + 64, t0 : t0 + 512]
            pspv = PS2.tile([128, 512], F32, tag="pspv", name="pspv")
            n_pv = 0
            for jp in jp_order:
                pss = PS1.tile([128, 1024], F32, tag="psS", name="psS")
                diag_b = jp == 2 * c + 1  # blocks 4c+2/4c+3: half the columns
                # of this pair are fully above the diagonal; don't exp them
                for u in range(2):
                    j = 2 * jp + u
                    nc.tensor.matmul(
                        pss[:, 512 * u : 512 * u + 512],
                        kTd[row : row + 64, 128 * j : 128 * j + 128],
                        q_ap,
                        start=True,
                        stop=True,
                    )
                pt = PT.tile(
                    [128, 2, 512], ptdt,
                    tag="pt" if use_dr else "ptb",
                    name="pt",
                )
                if diag_b:
                    nc.gpsimd.memset(pt[:, 0, 0:256], 0.0)
                    nc.gpsimd.memset(pt[:, 1, 0:384], 0.0)
                    nc.scalar.activation(
                        out=pt[:, 0, 256:512], in_=pss[:, 256:512],
                        func=AF.Exp, scale=0.125,
                    )
                    nc.scalar.activation(
                        out=pt[:, 1, 384:512], in_=pss[:, 896:1024],
                        func=AF.Exp, scale=0.125,
                    )
                    for u in range(2):
                        off = 256 + 128 * u
                        nc.gpsimd.affine_select(
                            out=pt[:, u, off : off + 128],
                            in_=pt[:, u, off : off + 128],
                            compare_op=ALU.is_ge,
                            fill=0.0,
                            base=0,
                            channel_multiplier=-1,
                            pattern=[[1, 128]],
                        )
                else:
                    nc.scalar.activation(
                        out=pt.rearrange("p a b -> p (a b)"), in_=pss,
                        func=AF.Exp, scale=0.125,
                    )
                    for u in range(2):
                        j = 2 * jp + u
                        if j >= 4 * c:  # diagonal block: zero out k > q
                            span = 128 * (j - 4 * c) + 128
                            nc.gpsimd.affine_select(
                                out=pt[:, u, 0:span],
                                in_=pt[:, u, 0:span],
                                compare_op=ALU.is_ge,
                                fill=0.0,
                                base=t0 - 128 * j,
                                channel_multiplier=-1,
                                pattern=[[1, span]],
                            )
                n_pv += 1
                if use_dr:
                    nc.tensor.matmul(
                        pspv[0 : D + 2, :],
                        v8[:, 2 * jp : 2 * jp + 2, 0 : D + 2],
                        pt,
                        start=(n_pv == 1),
                        stop=(n_pv == npair),
                        perf_mode=mybir.MatmulPerfMode.DoubleRow,
                        skip_group_check=True,
                    )
                else:
                    vsrc = vbf if (PV_MODE == "fp8dr") else v8
                    for u in range(2):
                        nc.tensor.matmul(
                            pspv[0 : D + 2, :],
                            vsrc[:, 2 * jp + u, 0 : D + 2],
                            pt[:, u, :],
                            start=(n_pv == 1 and u == 0),
                            stop=(n_pv == npair and u == 1),
                            skip_group_check=True,
                        )
            # normalize on-chip: 1/sums lives at partition 64; broadcast
            # into this pspv tile's spare rows 64:128 with a K=1 matmul
            rzt = RZ.tile([65, 512], BF16, tag="rzt", name="rzt")
            with nc.allow_low_precision("softmax sums are O(100)"):
                nc.vector.reciprocal(out=rzt[64:65, :], in_=pspv[64:65, :])
            nc.tensor.matmul(
                pspv[64:128, :], ones64[64:65, :], rzt[64:65, :],
                start=True, stop=True, skip_group_check=True,
            )
            # DVE may read only one PSUM operand: stage the broadcast
            zbs = ON.tile([D, 512], BF16, tag="zbs", name="zbs")
            nc.vector.tensor_copy(out=zbs, in_=pspv[64:128, :])
            otn = ON.tile([D, 512], BF16, tag="otn", name="otn")
            nc.vector.tensor_mul(otn, pspv[0:D, :], zbs)
            # re-pair heads across partitions with identity matmuls into
            # this head's pspv tile: even head rows 0:64, odd rows 64:128
            pb = h // 2
            if h % 2 == 0:
                state["otn_even"] = otn
            if h % 2 == 1 or h == HG - 1:
                solo = h % 2 == 0
                nc.tensor.matmul(
                    pspv[0:64, :], perm_sb[0:64, 256:320], state["otn_even"],
                    start=True, stop=True, skip_group_check=True,
                )
                if not solo:
                    nc.tensor.matmul(
                        pspv[64:128, :], perm_sb[0:64, 256:320], otn,
                        start=True, stop=True, skip_group_check=True,
                    )
                    nc.vector.tensor_copy(out=otl[:, pb, :], in_=pspv[0:128, :])
                else:
                    nc.vector.tensor_copy(out=otl[0:64, pb, :], in_=pspv[0:64, :])

        def make_otl(c):
            odt = FP8 if c > 0 else BF16
            otl = OL.tile(
                [128, 4, 512], odt, tag="otl8" if c > 0 else "otlb", name="otl"
            )
            if c > 0:
                nc.vector.memset(otl[64:128, 3, :], 0.0)
            return otl

        def emit_attention_multi(chunk_offsets, steps_by_round, otls):
            """Interleave several chunks' head loops, each starting at its
            round offset, with explicit per-round extra work so the exp
            stream always has queued score tiles."""
            for c in chunk_offsets:
                otls[c] = make_otl(c)
            states = {c: {} for c in chunk_offsets}
            nrounds = max(
                [off + HG for off in chunk_offsets.values()]
                + [len(steps_by_round)]
            )
            for r in range(nrounds):
                for c, off in chunk_offsets.items():
                    hi = r - off
                    if 0 <= hi < HG:
                        h = [6, 0, 1, 2, 3, 4, 5][hi] if c == 0 else hi
                        att_head(c, h, otls[c], states[c])
                if r < len(steps_by_round):
                    for fn in steps_by_round[r]:
                        fn()

        def _slab(c, s, ps1=False):
            return lambda: proj_slab(c, s, ps1)

        def _v(c, tb, ps1=False):
            return lambda: proj_v(c, tb, ps1)

        # proj(0) essentials up front (borrowing the idle score buffers):
        # heads 0/1 need slab 0, head 6 and all scores need kTd (slab 3), V
        for st0 in (
            [_slab(0, 3), _slab(0, 0)]
            + [_v(0, tb) for tb in range(4)]
        ):
            st0()
        otls = {}
        p2 = proj_steps(2)
        p3 = proj_steps(3)

        def _op(cc, tb):
            return lambda: oproj_tb(cc, otls[cc], tb)

        # one continuous pipeline: chunk c's heads start at its round offset,
        # with projections/o_proj token-blocks pinned to rounds that strictly
        # precede their consumers
        emit_attention_multi(
            {0: 0, 1: 2, 2: 6, 3: 11},
            [
                [_slab(1, 3), _slab(0, 1), _v(1, 0), _v(1, 1)],
                [_slab(1, 0), _slab(0, 2), _v(1, 2), _v(1, 3)],
                [_slab(1, 1), _slab(1, 2)],
                [p2[0], p2[1], p2[4], p2[5]],
                [p2[2], p2[3], p2[6], p2[7]],
                [],
                [],
                [p3[0], p3[1], p3[4], p3[5]],
                [p3[2], p3[3], p3[6], p3[7]],
                [_op(0, 0), _op(0, 1)],
                [_op(0, 2), _op(0, 3)],
                [_op(1, 0), _op(1, 1)],
                [_op(1, 2), _op(1, 3)],
                [_op(2, 0), _op(2, 1)],
                [_op(2, 2), _op(2, 3)],
            ],
            otls,
        )
        for tb in range(4):
            oproj_tb(3, otls[3], tb, use_ps1=True)

    nc.finalize()
    return nc


def _bf16(x):
    return np.ascontiguousarray(x).astype(_BF16)


def _fp8(x):
    return np.ascontiguousarray(x).astype(ml_dtypes.float8_e4m3fn)


def _prep_core(hidden, q_w, q_b, k_w, k_b, v_w, v_b, o_w, pos, b, g):
    hseq = hidden[S * b : S * (b + 1)]  # [S, HID]
    hT = np.ascontiguousarray(hseq.T).reshape(KBLK, 128, S)

    qg = q_w[:, NQ * g : NQ * (g + 1)]  # [HID, 448]
    kg = k_w[:, D * g : D * (g + 1)]  # [HID, 64]
    qk = np.concatenate([qg, kg], axis=1)  # [HID, 512]
    wqk_ = np.ascontiguousarray(qk).reshape(KBLK, 128, NQK)

    bq = np.concatenate([q_b[NQ * g : NQ * (g + 1)], k_b[D * g : D * (g + 1)]])
    bqk_ = np.ascontiguousarray(bq.reshape(NSLAB, 128).T)

    wv_ = np.ascontiguousarray(v_w[:, D * g : D * (g + 1)]).reshape(KBLK, 128, D)
    vb_ = np.concatenate(
        [v_b[D * g : D * (g + 1)], np.ones(2, np.float32)]
    ).reshape(1, D + 2)
    # o_proj weights: [448, HID] -> 4 partition blocks (last padded 64->128)
    ows = np.zeros((4, 128, HID), np.float32)
    ows.reshape(512, HID)[0:NQ] = o_w[NQ * g : NQ * (g + 1), :]

    p = pos[S * b : S * (b + 1)].astype(np.float32)
    inv_freq = 1.0 / (THETA ** (np.arange(0, D, 2, dtype=np.float32) / D))  # [32]
    ang = inv_freq[:, None] * p[None, :]  # [32, S]
    cos = np.ascontiguousarray(np.tile(np.cos(ang), (4, 1)))  # [128, S]
    sinpat_ = np.ascontiguousarray(np.tile(np.sin(ang), (4, 1)))  # [128, S]

    # perm[:, 0:128]: sign-folded rotate_half (block-diag per 64);
    # perm[:, 128:256]: duplicate rows 64:128 into both halves (for kTd);
    # perm[:, 256:320]: identity (rows 0:64) for the o_proj head re-pairing
    rot64 = np.zeros((64, 64), np.float32)
    for m in range(32):
        rot64[m + 32, m] = -1.0
        rot64[m, m + 32] = 1.0
    rblk = np.zeros((128, 128), np.float32)
    rblk[0:64, 0:64] = rot64
    rblk[64:128, 64:128] = rot64
    dup = np.zeros((128, 128), np.float32)
    for m in range(64):
        dup[64 + m, m] = 1.0
        dup[64 + m, 64 + m] = 1.0
    ident = np.zeros((128, 64), np.float32)
    ident[0:64, 0:64] = np.eye(64, dtype=np.float32)
    perm_ = np.ascontiguousarray(np.concatenate([rblk, dup, ident], axis=1))

    return {
        "hT": _fp8(hT),
        "hb0": _bf16(hT[:, :, 0:512]),
        "wqkb": _bf16(wqk_),
        "wvb": _bf16(wv_),
        "wqk": _fp8(wqk_),
        "wv": _fp8(wv_),
        "bqk": bqk_.astype(np.float32),
        "vb": _bf16(vb_),
        "ow": _fp8(ows),
        "owb": _bf16(ows),
        "cosf": _bf16(cos),
        "sinpat": _bf16(sinpat_),
        "perm": _bf16(perm_),
    }


def kernel(hidden_states, q_w, q_b, k_w, k_b, v_w, v_b, o_w, position_ids):
    hidden_states = np.asarray(hidden_states, dtype=np.float32)
    q_w = np.asarray(q_w, dtype=np.float32)
    q_b = np.asarray(q_b, dtype=np.float32)
    k_w = np.asarray(k_w, dtype=np.float32)
    k_b = np.asarray(k_b, dtype=np.float32)
    v_w = np.asarray(v_w, dtype=np.float32)
    v_b = np.asarray(v_b, dtype=np.float32)
    o_w = np.asarray(o_w, dtype=np.float32)
    position_ids = np.asarray(position_ids)

    if "nc" not in _CACHE:
        _CACHE["nc"] = _build()
    nc = _CACHE["nc"]

    in_maps = []
    for c in range(N_CORES):
        b, g = c // 2, c % 2
        in_maps.append(
            _prep_core(
                hidden_states, q_w, q_b, k_w, k_b, v_w, v_b, o_w, position_ids, b, g
            )
        )

    res = run_bass_kernel_spmd(nc, in_maps, core_ids=list(range(N_CORES)))
    parts = [np.asarray(r["out"], dtype=np.float32) for r in res.results]
    return np.concatenate(
        [parts[2 * b] + parts[2 * b + 1] for b in range(B)], axis=0
    ).astype(np.float32)


if __name__ == "__main__":
    rng = np.random.default_rng(0)
    T = B * S
    ins = {
        "hidden_states": rng.standard_normal((T, HID)).astype(np.float32),
        "q_w": (rng.standard_normal((HID, HID)) * 0.02).astype(np.float32),
        "q_b": (rng.standard_normal((HID,)) * 0.02).astype(np.float32),
        "k_w": (rng.standard_normal((HID, KV * D)) * 0.02).astype(np.float32),
        "k_b": (rng.standard_normal((KV * D,)) * 0.02).astype(np.float32),
        "v_w": (rng.standard_normal((HID, KV * D)) * 0.02).astype(np.float32),
        "v_b": (rng.standard_normal((KV * D,)) * 0.02).astype(np.float32),
        "o_w": (rng.standard_normal((HID, HID)) * 0.02).astype(np.float32),
        "position_ids": np.tile(np.arange(S, dtype=np.int32), B),
    }
    out = kernel(**ins)
    print("kernel output", out.shape, out.dtype, np.abs(out).max())


# revision 5
# speedup vs baseline: 1.0381x; 1.0028x over previous
"""Trainium2 Bass kernel for Qwen2-style causal self-attention (GQA + RoPE).

Geometry: B=4 seqs x S=2048 tokens, 14 Q heads / 2 KV heads, D=64, HID=896.
Sharding: 8 cores = 4 sequences x 2 head-groups (7 Q heads + 1 KV head each).
Each core computes its sequence's QKV projections (its head shard), RoPE,
causal attention, and a partial o_proj (448 input dims); the host sums the
two partials per sequence.

Design (driven by the TimelineSim cost model + real-HW numerics):
  - ScalarE is the bottleneck engine and does exp only (~139us); all
    scheduling aims to keep score PSUM tiles queued for it.
  - One continuous software pipeline: every chunk's head loop starts at a
    round offset and the projection / o_proj work is pinned to rounds that
    strictly precede its consumers, so each engine FIFO alternates between
    pipelines instead of head-of-line blocking on a phase.
  - fp8e4 DoubleRow (K=256/instruction, 0.5 cycles/row) for the QKV
    projections, PV, and o_proj of chunks 1-3; probs are quantized to fp8
    by the exp activation itself.  Chunk 0 stays bf16 end-to-end: a dot
    product's relative error does not average down with contraction size,
    and early tokens (few keys, large outputs) dominate the error budget.
  - causal masking on GPSIMD affine_select over the affected span only;
    the half of each second diagonal pair that is fully masked is never
    exp'd (GPSIMD zero-fills it instead).
  - softmax normalization per head stays on-chip: reciprocal of the [V|1]
    ones-column sums at partition 64, broadcast down via a K=1 matmul into
    the same PSUM tile's spare rows, staged once through SBUF (DVE may
    read only one PSUM operand), one multiply.
  - heads re-paired for o_proj (even head -> partitions 0:64, odd ->
    64:128) with identity matmuls into the same PSUM tile (both matmuls
    start=True: a start only clears its own region's has_written bits).
  - PSUM: scores 2x2 banks, pspv 2x1, proj 2x1 (each slab reuses one tile
    for the proj/rotate/dup outputs), o_proj shares the proj tag; the
    final o_proj borrows the then-idle score buffers.
  - PE p-state warmup matmuls run while the startup DMAs stream; DMAs are
    ordered/split by criticality (slab-0/3 weight columns, chunk-0 hidden
    pieces, chunk-0 cos/sin columns first).
"""

import numpy as np
from contextlib import ExitStack

import concourse.bacc as bacc
import concourse.bass as bass
import concourse.mybir as mybir
import concourse.tile as tile
from concourse.bass_utils import run_bass_kernel_spmd

import ml_dtypes

_BF16 = ml_dtypes.bfloat16

B, S = 4, 2048
H, KV, D = 14, 2, 64
HID = H * D  # 896
THETA = 1000000.0
G = 2  # tensor-parallel head groups
HG = H // G  # 7 q heads per group
NQ = HG * D  # 448
NQK = NQ + D  # 512 = q dims + k dims per group
KBLK = HID // 128  # 7 hid blocks
NSLAB = NQK // 128  # 4 slabs of the roped qk output
NTOK = S // 128  # 16 token blocks
NCHUNK = S // 512  # 4 token chunks
N_CORES = 8

F32 = mybir.dt.float32
BF16 = mybir.dt.bfloat16
FP8 = mybir.dt.float8e4
AF = mybir.ActivationFunctionType
ALU = mybir.AluOpType

PV_MODE = "fp8dr"  # "fp8dr" | "bf16"

_CACHE = {}


def _build():
    nc = bacc.Bacc("TRN2", target_bir_lowering=False, debug=False)

    hT = nc.dram_tensor("hT", [KBLK, 128, S], FP8, kind="ExternalInput")
    wqk = nc.dram_tensor("wqk", [KBLK, 128, NQK], FP8, kind="ExternalInput")
    wv = nc.dram_tensor("wv", [KBLK, 128, D], FP8, kind="ExternalInput")
    hb0 = nc.dram_tensor("hb0", [KBLK, 128, 512], BF16, kind="ExternalInput")
    wqkb = nc.dram_tensor("wqkb", [KBLK, 128, NQK], BF16, kind="ExternalInput")
    wvb = nc.dram_tensor("wvb", [KBLK, 128, D], BF16, kind="ExternalInput")
    bqk = nc.dram_tensor("bqk", [128, NSLAB], F32, kind="ExternalInput")
    vb = nc.dram_tensor("vb", [1, D + 2], BF16, kind="ExternalInput")
    ow = nc.dram_tensor("ow", [4, 128, HID], FP8, kind="ExternalInput")
    owb = nc.dram_tensor("owb", [4, 128, HID], BF16, kind="ExternalInput")
    cosf = nc.dram_tensor("cosf", [128, S], BF16, kind="ExternalInput")
    sinpat = nc.dram_tensor("sinpat", [128, S], BF16, kind="ExternalInput")
    perm = nc.dram_tensor("perm", [128, 320], BF16, kind="ExternalInput")
    out = nc.dram_tensor("out", [S, HID], BF16, kind="ExternalOutput")

    vdt = FP8 if PV_MODE == "fp8dr" else BF16

    with tile.TileContext(nc) as tc, ExitStack() as ctx:
        P = ctx.enter_context(tc.tile_pool(name="persist", bufs=1))
        RT = ctx.enter_context(tc.tile_pool(name="rt", bufs=6))
        PT = ctx.enter_context(tc.tile_pool(name="pt", bufs=12))
        RZ = ctx.enter_context(tc.tile_pool(name="rz", bufs=8))
        ON = ctx.enter_context(tc.tile_pool(name="on", bufs=12))
        OL = ctx.enter_context(tc.tile_pool(name="ol", bufs=5))
        OB = ctx.enter_context(tc.tile_pool(name="ob", bufs=6))
        PS1 = ctx.enter_context(tc.tile_pool(name="ps1", bufs=2, space="PSUM"))
        PS2 = ctx.enter_context(tc.tile_pool(name="ps2", bufs=2, space="PSUM"))
        PS3 = ctx.enter_context(tc.tile_pool(name="ps3", bufs=2, space="PSUM"))

        # ---- persistent tiles ----
        qk_sb = [P.tile([128, S], BF16, tag=f"qk{s}", name=f"qk{s}") for s in range(NSLAB)]
        kTd = P.tile([128, S], BF16, tag="kTd")
        cos_sb = P.tile([128, S], BF16, tag="cos")
        sin_sb = P.tile([128, S], BF16, tag="sin")
        bqk_sb = P.tile([128, NSLAB], F32, tag="bqk")
        wqk_sb = P.tile([128, KBLK, NQK], FP8, tag="wqk_sb")
        wv_sb = P.tile([128, KBLK, D], FP8, tag="wv_sb")
        ow_sb = P.tile([128, 4, HID], FP8, tag="ow_sb")
        owb_sb = P.tile([128, 4, HID], BF16, tag="owb_sb")
        vb_sb = P.tile([1, D + 2], BF16, tag="vb_sb")
        ones1 = P.tile([1, 128], BF16, tag="ones1")
        perm_sb = P.tile([128, 320], BF16, tag="perm_sb")
        v8 = P.tile([128, NTOK, 80], vdt, tag="v8")
        vbf = P.tile([128, 4, 80], BF16, tag="vbf")  # chunk-0 V in bf16
        h_sb = P.tile([128, KBLK, S], FP8, tag="h_sb")  # full hidden, resident
        hb0_sb = P.tile([128, KBLK, 512], BF16, tag="hb0_sb")  # chunk-0 hidden
        wqkb_sb = P.tile([128, KBLK, NQK], BF16, tag="wqkb_sb")
        wvb_sb = P.tile([128, KBLK, D], BF16, tag="wvb_sb")
        ones64 = P.tile([65, D], BF16, tag="ones64")
        ones512 = P.tile([1, 512], BF16, tag="ones512")

        # warm the PE p-state while the startup DMAs stream: the cost model
        # runs matmuls at half rate until ~3us of continuous PE activity
        nc.vector.memset(ones1, 1.0)
        nc.vector.memset(ones512, 1.0)
        for _w in range(8):
            wps = PS3.tile([128, 512], F32, tag="psA", name="warm")
            nc.tensor.matmul(wps, ones1, ones512, start=True, stop=True)

        # startup loads, most critical first: proj(0) slab 3 (K) needs the
        # slab-3 weight columns + chunk-0 hidden + bias + perm + cos/sin
        nc.sync.dma_start(
            out=wqkb_sb[:, :, 384:512],
            in_=wqkb[:, :, 384:512].rearrange("k p m -> p k m"),
        )
        nc.sync.dma_start(
            out=wqkb_sb[:, :, 0:128],
            in_=wqkb[:, :, 0:128].rearrange("k p m -> p k m"),
        )
        for k0, k1 in ((0, 2), (2, 4), (4, 6), (6, 7)):
            nc.sync.dma_start(
                out=hb0_sb[:, k0:k1, :],
                in_=hb0[k0:k1, :, :].rearrange("k p t -> p k t"),
            )
        nc.sync.dma_start(out=bqk_sb, in_=bqk[:, :])
        nc.sync.dma_start(out=perm_sb, in_=perm[:, :])
        nc.sync.dma_start(out=cos_sb[:, 0:512], in_=cosf[:, 0:512])
        nc.sync.dma_start(out=sin_sb[:, 0:512], in_=sinpat[:, 0:512])
        nc.sync.dma_start(out=cos_sb[:, 512:S], in_=cosf[:, 512:S])
        nc.sync.dma_start(out=sin_sb[:, 512:S], in_=sinpat[:, 512:S])
        nc.sync.dma_start(
            out=wqkb_sb[:, :, 128:384],
            in_=wqkb[:, :, 128:384].rearrange("k p m -> p k m"),
        )
        nc.sync.dma_start(out=wvb_sb, in_=wvb[:, :, :].rearrange("k p m -> p k m"))
        nc.sync.dma_start(out=wqk_sb, in_=wqk[:, :, :].rearrange("k p m -> p k m"))
        nc.sync.dma_start(out=wv_sb, in_=wv[:, :, :].rearrange("k p m -> p k m"))
        nc.sync.dma_start(out=vb_sb, in_=vb[:, :])
        for c in range(1, NCHUNK):
            nc.sync.dma_start(
                out=h_sb[:, :, 512 * c : 512 * c + 512],
                in_=hT[:, :, 512 * c : 512 * c + 512].rearrange("k p t -> p k t"),
            )
        nc.sync.dma_start(out=ow_sb, in_=ow[:, :, :].rearrange("b p m -> p b m"))
        nc.sync.dma_start(out=owb_sb, in_=owb[:, :, :].rearrange("b p m -> p b m"))
        nc.vector.memset(ones64, 1.0)

        def psa(use_ps1):
            if use_ps1:
                return PS1.tile([128, 1024], F32, tag="psS", name="psA")[:, 0:512]
            return PS3.tile([128, 512], F32, tag="psA", name="psA")

        def proj_slab(c, s, use_ps1=False):
            t0 = 512 * c
            ps = psa(use_ps1)
            if c == 0:
                # chunk-0 keys/values feed every later chunk and its outputs
                # are single-value-dominated: full bf16 projection
                for k in range(KBLK):
                    nc.tensor.matmul(
                        ps,
                        wqkb_sb[:, k, 128 * s : 128 * s + 128],
                        hb0_sb[:, k, :],
                        start=(k == 0),
                        stop=(k == KBLK - 1),
                    )
            else:
                h_t = h_sb[:, :, t0 : t0 + 512]
                for p in range(3):  # contraction pairs of hid blocks (fp8 DR)
                    nc.tensor.matmul(
                        ps,
                        wqk_sb[:, 2 * p : 2 * p + 2, 128 * s : 128 * s + 128],
                        h_t[:, 2 * p : 2 * p + 2, :],
                        start=(p == 0),
                        stop=False,
                        perf_mode=mybir.MatmulPerfMode.DoubleRow,
                        skip_group_check=True,
                    )
                nc.tensor.matmul(
                    ps,
                    wqk_sb[:, 6, 128 * s : 128 * s + 128],
                    h_t[:, 6, :],
                    start=False,
                    stop=True,
                    skip_group_check=True,
                )
            q = qk_sb[s][:, t0 : t0 + 512]
            nc.vector.tensor_scalar_add(q, ps, bqk_sb[:, s : s + 1])
            # rotate_half via sign-folded permutation matmul (cross-partition);
            # the slab's PSUM tile is reused for the rotate and dup outputs
            nc.tensor.matmul(ps, perm_sb[:, 0:128], q, start=True, stop=True)
            r = RT.tile([128, 512], BF16, tag="qkr", name="qkr")
            nc.vector.tensor_mul(r, ps, sin_sb[:, t0 : t0 + 512])
            nc.vector.tensor_mul(q, q, cos_sb[:, t0 : t0 + 512])
            nc.vector.tensor_add(q, q, r)
            if s == NSLAB - 1:
                # duplicated roped K^T (both partition halves)
                nc.tensor.matmul(ps, perm_sb[:, 128:256], q, start=True, stop=True)
                nc.vector.tensor_copy(out=kTd[:, t0 : t0 + 512], in_=ps)

        def proj_v(c, tb, use_ps1=False):
            t0 = 512 * c
            h_t = h_sb[:, :, t0 : t0 + 512]
            t = 4 * c + tb
            psv = psa(use_ps1)[:, 0 : D + 2]
            nc.tensor.matmul(
                psv, ones1, vb_sb, start=True, stop=False, skip_group_check=True
            )
            if c == 0:
                for k in range(KBLK):
                    nc.tensor.matmul(
                        psv[:, 0:D],
                        hb0_sb[:, k, 128 * tb : 128 * tb + 128],
                        wvb_sb[:, k, :],
                        start=False,
                        stop=(k == KBLK - 1),
                        skip_group_check=True,
                    )
            else:
                for p in range(3):
                    nc.tensor.matmul(
                        psv[:, 0:D],
                        h_t[:, 2 * p : 2 * p + 2, 128 * tb : 128 * tb + 128],
                        wv_sb[:, 2 * p : 2 * p + 2, :],
                        start=False,
                        stop=False,
                        perf_mode=mybir.MatmulPerfMode.DoubleRow,
                        skip_group_check=True,
                    )
                nc.tensor.matmul(
                    psv[:, 0:D],
                    h_t[:, 6, 128 * tb : 128 * tb + 128],
                    wv_sb[:, 6, :],
                    start=False,
                    stop=True,
                    skip_group_check=True,
                )
            nc.vector.tensor_copy(out=v8[:, t, 0 : D + 2], in_=psv)
            if c == 0:
                nc.vector.tensor_copy(out=vbf[:, t, 0 : D + 2], in_=psv)

        def proj_steps(c, use_ps1=False):
            steps = [lambda s=s: proj_slab(c, s, use_ps1) for s in (3, 0, 1, 2)]
            steps += [lambda tb=tb: proj_v(c, tb, use_ps1) for tb in range(4)]
            return steps

        def oproj_tb(c, otl, tb, use_ps1=False):
            t = 4 * c + tb
            ob = OB.tile([128, HID], BF16, tag="ob", name="ob")
            for n0, n1 in ((0, 512), (512, HID)):
                if use_ps1:
                    po = PS1.tile([128, 1024], F32, tag="psS", name="po")[:, 0:512]
                else:
                    po = PS3.tile([128, 512], F32, tag="psA", name="po")
                if c > 0:
                    # fp8 DR: contract head-pair blocks two at a time (K=256)
                    for i in range(2):
                        nc.tensor.matmul(
                            po[:, 0 : n1 - n0],
                            otl[:, 2 * i : 2 * i + 2, 128 * tb : 128 * tb + 128],
                            ow_sb[:, 2 * i : 2 * i + 2, n0:n1],
                            start=(i == 0),
                            stop=(i == 1),
                            perf_mode=mybir.MatmulPerfMode.DoubleRow,
                            skip_group_check=True,
                        )
                else:
                    for pb in range(4):
                        p_n = 128 if pb < 3 else 64
                        nc.tensor.matmul(
                            po[:, 0 : n1 - n0],
                            otl[0:p_n, pb, 128 * tb : 128 * tb + 128],
                            owb_sb[0:p_n, pb, n0:n1],
                            start=(pb == 0),
                            stop=(pb == 3),
                        )
                if use_ps1 and n0 == 0:
                    nc.scalar.copy(out=ob[:, n0:n1], in_=po[:, 0 : n1 - n0])
                else:
                    nc.vector.tensor_copy(
                        out=ob[:, n0:n1], in_=po[:, 0 : n1 - n0]
                    )
            nc.sync.dma_start(out=out[128 * t : 128 * t + 128, :], in_=ob)

        def att_head(c, h, otl, state):
            t0 = 512 * c
            npair = 2 * c + 2
            use_dr = PV_MODE == "fp8dr" and c > 0
            ptdt = vdt if use_dr else BF16
            # diagonal pairs first so gpsimd masking overlaps later blocks
            jp_order = [2 * c, 2 * c + 1] + list(range(0, 2 * c))
            slab = h // 2
            row = 64 * (h % 2)
            q_ap = qk_sb[slab][row : row + 64, t0 : t0 + 512]
            pspv = PS2.tile([128, 512], F32, tag="pspv", name="pspv")
            n_pv = 0
            for jp in jp_order:
                pss = PS1.tile([128, 1024], F32, tag="psS", name="psS")
                diag_b = jp == 2 * c + 1  # blocks 4c+2/4c+3: half the columns
                # of this pair are fully above the diagonal; don't exp them
                for u in range(2):
                    j = 2 * jp + u
                    nc.tensor.matmul(
                        pss[:, 512 * u : 512 * u + 512],
                        kTd[row : row + 64, 128 * j : 128 * j + 128],
                        q_ap,
                        start=True,
                        stop=True,
                    )
                pt = PT.tile(
                    [128, 2, 512], ptdt,
                    tag="pt" if use_dr else "ptb",
                    name="pt",
                )
                if diag_b:
                    nc.gpsimd.memset(pt[:, 0, 0:256], 0.0)
                    nc.gpsimd.memset(pt[:, 1, 0:384], 0.0)
                    nc.scalar.activation(
                        out=pt[:, 0, 256:512], in_=pss[:, 256:512],
                        func=AF.Exp, scale=0.125,
                    )
                    nc.scalar.activation(
                        out=pt[:, 1, 384:512], in_=pss[:, 896:1024],
                        func=AF.Exp, scale=0.125,
                    )
                    for u in range(2):
                        off = 256 + 128 * u
                        nc.gpsimd.affine_select(
                            out=pt[:, u, off : off + 128],
                            in_=pt[:, u, off : off + 128],
                            compare_op=ALU.is_ge,
                            fill=0.0,
                            base=0,
                            channel_multiplier=-1,
                            pattern=[[1, 128]],
                        )
                else:
                    nc.scalar.activation(
                        out=pt.rearrange("p a b -> p (a b)"), in_=pss,
                        func=AF.Exp, scale=0.125,
                    )
                    for u in range(2):
                        j = 2 * jp + u
                        if j >= 4 * c:  # diagonal block: zero out k > q
                            span = 128 * (j - 4 * c) + 128
                            nc.gpsimd.affine_select(
                                out=pt[:, u, 0:span],
                                in_=pt[:, u, 0:span],
                                compare_op=ALU.is_ge,
                                fill=0.0,
                                base=t0 - 128 * j,
                                channel_multiplier=-1,
                                pattern=[[1, span]],
                            )
                n_pv += 1
                if use_dr:
                    nc.tensor.matmul(
                        pspv[0 : D + 2, :],
                        v8[:, 2 * jp : 2 * jp + 2, 0 : D + 2],
                        pt,
                        start=(n_pv == 1),
                        stop=(n_pv == npair),
                        perf_mode=mybir.MatmulPerfMode.DoubleRow,
                        skip_group_check=True,
                    )
                else:
                    vsrc = vbf if (PV_MODE == "fp8dr") else v8
                    for u in range(2):
                        nc.tensor.matmul(
                            pspv[0 : D + 2, :],
                            vsrc[:, 2 * jp + u, 0 : D + 2],
                            pt[:, u, :],
                            start=(n_pv == 1 and u == 0),
                            stop=(n_pv == npair and u == 1),
                            skip_group_check=True,
                        )
            # normalize on-chip: 1/sums lives at partition 64; broadcast
            # into this pspv tile's spare rows 64:128 with a K=1 matmul
            rzt = RZ.tile([65, 512], BF16, tag="rzt", name="rzt")
            with nc.allow_low_precision("softmax sums are O(100)"):
                nc.vector.reciprocal(out=rzt[64:65, :], in_=pspv[64:65, :])
            nc.tensor.matmul(
                pspv[64:128, :], ones64[64:65, :], rzt[64:65, :],
                start=True, stop=True, skip_group_check=True,
            )
            # DVE may read only one PSUM operand: stage the broadcast
            zbs = ON.tile([D, 512], BF16, tag="zbs", name="zbs")
            nc.vector.tensor_copy(out=zbs, in_=pspv[64:128, :])
            otn = ON.tile([D, 512], BF16, tag="otn", name="otn")
            nc.vector.tensor_mul(otn, pspv[0:D, :], zbs)
            # re-pair heads across partitions with identity matmuls into
            # this head's pspv tile: even head rows 0:64, odd rows 64:128
            pb = h // 2
            if h % 2 == 0:
                state["otn_even"] = otn
            if h % 2 == 1 or h == HG - 1:
                solo = h % 2 == 0
                nc.tensor.matmul(
                    pspv[0:64, :], perm_sb[0:64, 256:320], state["otn_even"],
                    start=True, stop=True, skip_group_check=True,
                )
                if not solo:
                    nc.tensor.matmul(
                        pspv[64:128, :], perm_sb[0:64, 256:320], otn,
                        start=True, stop=True, skip_group_check=True,
                    )
                    nc.vector.tensor_copy(out=otl[:, pb, :], in_=pspv[0:128, :])
                else:
                    nc.vector.tensor_copy(out=otl[0:64, pb, :], in_=pspv[0:64, :])

        def make_otl(c):
            odt = FP8 if c > 0 else BF16
            otl = OL.tile(
                [128, 4, 512], odt, tag="otl8" if c > 0 else "otlb", name="otl"
            )
            if c > 0:
                nc.vector.memset(otl[64:128, 3, :], 0.0)
            return otl

        def emit_attention_multi(chunk_offsets, steps_by_round, otls):
            """Interleave several chunks' head loops, each starting at its
            round offset, with explicit per-round extra work so the exp
            stream always has queued score tiles."""
            for c in chunk_offsets:
                otls[c] = make_otl(c)
            states = {c: {} for c in chunk_offsets}
            nrounds = max(
                [off + HG for off in chunk_offsets.values()]
                + [len(steps_by_round)]
            )
            for r in range(nrounds):
                for c, off in chunk_offsets.items():
                    hi = r - off
                    if 0 <= hi < HG:
                        h = [6, 0, 1, 2, 3, 4, 5][hi] if c == 0 else hi
                        att_head(c, h, otls[c], states[c])
                if r < len(steps_by_round):
                    for fn in steps_by_round[r]:
                        fn()

        def _slab(c, s, ps1=False):
            return lambda: proj_slab(c, s, ps1)

        def _v(c, tb, ps1=False):
            return lambda: proj_v(c, tb, ps1)

        # proj(0) essentials up front (borrowing the idle score buffers):
        # heads 0/1 need slab 0, head 6 and all scores need kTd (slab 3), V
        for st0 in (
            [_slab(0, 0), _slab(0, 3)]
            + [_v(0, tb) for tb in range(4)]
        ):
            st0()
        otls = {}
        p2 = proj_steps(2)
        p3 = proj_steps(3)

        def _op(cc, tb):
            return lambda: oproj_tb(cc, otls[cc], tb)

        # one continuous pipeline: chunk c's heads start at its round offset,
        # with projections/o_proj token-blocks pinned to rounds that strictly
        # precede their consumers
        emit_attention_multi(
            {0: 0, 1: 2, 2: 6, 3: 11},
            [
                [_slab(1, 3), _slab(0, 1), _v(1, 0), _v(1, 1)],
                [_slab(1, 0), _slab(0, 2), _v(1, 2), _v(1, 3)],
                [_slab(1, 1), _slab(1, 2)],
                [p2[0], p2[1], p2[4], p2[5]],
                [p2[2], p2[3], p2[6], p2[7]],
                [],
                [],
                [p3[0], p3[1], p3[4], p3[5]],
                [p3[2], p3[3], p3[6], p3[7]],
                [_op(0, 0), _op(0, 1)],
                [_op(0, 2), _op(0, 3)],
                [_op(1, 0), _op(1, 1)],
                [_op(1, 2), _op(1, 3)],
                [_op(2, 0), _op(2, 1)],
                [_op(2, 2), _op(2, 3)],
            ],
            otls,
        )
        for tb in range(4):
            oproj_tb(3, otls[3], tb, use_ps1=True)

    nc.finalize()
    return nc


def _bf16(x):
    return np.ascontiguousarray(x).astype(_BF16)


def _fp8(x):
    return np.ascontiguousarray(x).astype(ml_dtypes.float8_e4m3fn)


def _prep_core(hidden, q_w, q_b, k_w, k_b, v_w, v_b, o_w, pos, b, g):
    hseq = hidden[S * b : S * (b + 1)]  # [S, HID]
    hT = np.ascontiguousarray(hseq.T).reshape(KBLK, 128, S)

    qg = q_w[:, NQ * g : NQ * (g + 1)]  # [HID, 448]
    kg = k_w[:, D * g : D * (g + 1)]  # [HID, 64]
    qk = np.concatenate([qg, kg], axis=1)  # [HID, 512]
    wqk_ = np.ascontiguousarray(qk).reshape(KBLK, 128, NQK)

    bq = np.concatenate([q_b[NQ * g : NQ * (g + 1)], k_b[D * g : D * (g + 1)]])
    bqk_ = np.ascontiguousarray(bq.reshape(NSLAB, 128).T)

    wv_ = np.ascontiguousarray(v_w[:, D * g : D * (g + 1)]).reshape(KBLK, 128, D)
    vb_ = np.concatenate(
        [v_b[D * g : D * (g + 1)], np.ones(2, np.float32)]
    ).reshape(1, D + 2)
    # o_proj weights: [448, HID] -> 4 partition blocks (last padded 64->128)
    ows = np.zeros((4, 128, HID), np.float32)
    ows.reshape(512, HID)[0:NQ] = o_w[NQ * g : NQ * (g + 1), :]

    p = pos[S * b : S * (b + 1)].astype(np.float32)
    inv_freq = 1.0 / (THETA ** (np.arange(0, D, 2, dtype=np.float32) / D))  # [32]
    ang = inv_freq[:, None] * p[None, :]  # [32, S]
    cos = np.ascontiguousarray(np.tile(np.cos(ang), (4, 1)))  # [128, S]
    sinpat_ = np.ascontiguousarray(np.tile(np.sin(ang), (4, 1)))  # [128, S]

    # perm[:, 0:128]: sign-folded rotate_half (block-diag per 64);
    # perm[:, 128:256]: duplicate rows 64:128 into both halves (for kTd);
    # perm[:, 256:320]: identity (rows 0:64) for the o_proj head re-pairing
    rot64 = np.zeros((64, 64), np.float32)
    for m in range(32):
        rot64[m + 32, m] = -1.0
        rot64[m, m + 32] = 1.0
    rblk = np.zeros((128, 128), np.float32)
    rblk[0:64, 0:64] = rot64
    rblk[64:128, 64:128] = rot64
    dup = np.zeros((128, 128), np.float32)
    for m in range(64):
        dup[64 + m, m] = 1.0
        dup[64 + m, 64 + m] = 1.0
    ident = np.zeros((128, 64), np.float32)
    ident[0:64, 0:64] = np.eye(64, dtype=np.float32)
    perm_ = np.ascontiguousarray(np.concatenate([rblk, dup, ident], axis=1))

    return {
        "hT": _fp8(hT),
        "hb0": _bf16(hT[:, :, 0:512]),
        "wqkb": _bf16(wqk_),
        "wvb": _bf16(wv_),
        "wqk": _fp8(wqk_),
        "wv": _fp8(wv_),
        "bqk": bqk_.astype(np.float32),
        "vb": _bf16(vb_),
        "ow": _fp8(ows),
        "owb": _bf16(ows),
        "cosf": _bf16(cos),
        "sinpat": _bf16(sinpat_),
        "perm": _bf16(perm_),
    }


def kernel(hidden_states, q_w, q_b, k_w, k_b, v_w, v_b, o_w, position_ids):
    hidden_states = np.asarray(hidden_states, dtype=np.float32)
    q_w = np.asarray(q_w, dtype=np.float32)
    q_b = np.asarray(q_b, dtype=np.float32)
    k_w = np.asarray(k_w, dtype=np.float32)
    k_b = np.asarray(k_b, dtype=np.float32)
    v_w = np.asarray(v_w, dtype=np.float32)
    v_b = np.asarray(v_b, dtype=np.float32)
    o_w = np.asarray(o_w, dtype=np.float32)
    position_ids = np.asarray(position_ids)

    if "nc" not in _CACHE:
        _CACHE["nc"] = _build()
    nc = _CACHE["nc"]

    in_maps = []
    for c in range(N_CORES):
        b, g = c // 2, c % 2
        in_maps.append(
            _prep_core(
                hidden_states, q_w, q_b, k_w, k_b, v_w, v_b, o_w, position_ids, b, g
            )
        )

    res = run_bass_kernel_spmd(nc, in_maps, core_ids=list(range(N_CORES)))
    parts = [np.asarray(r["out"], dtype=np.float32) for r in res.results]
    return np.concatenate(
        [parts[2 * b] + parts[2 * b + 1] for b in range(B)], axis=0
    ).astype(np.float32)


if __name__ == "__main__":
    rng = np.random.default_rng(0)
    T = B * S
    ins = {
        "hidden_states": rng.standard_normal((T, HID)).astype(np.float32),
        "q_w": (rng.standard_normal((HID, HID)) * 0.02).astype(np.float32),
        "q_b": (rng.standard_normal((HID,)) * 0.02).astype(np.float32),
        "k_w": (rng.standard_normal((HID, KV * D)) * 0.02).astype(np.float32),
        "k_b": (rng.standard_normal((KV * D,)) * 0.02).astype(np.float32),
        "v_w": (rng.standard_normal((HID, KV * D)) * 0.02).astype(np.float32),
        "v_b": (rng.standard_normal((KV * D,)) * 0.02).astype(np.float32),
        "o_w": (rng.standard_normal((HID, HID)) * 0.02).astype(np.float32),
        "position_ids": np.tile(np.arange(S, dtype=np.int32), B),
    }
    out = kernel(**ins)
    print("kernel output", out.shape, out.dtype, np.abs(out).max())
